# revision 1
# baseline (speedup 1.0000x reference)
"""Trainium2 Bass kernel for MultiHeadAttention with RoPE + summed relative bias.

Reference computation (B=8, L=512, D=512, H=8, dh=64):
    Q,K,V = x @ W{q,k,v}.T + b ; RoPE(Q,K) (concat variant)
    scores = Q K^T / 8 + rel_bias.sum(-1)   (bias broadcast over batch+heads)
    out = softmax(scores) V @ Wo.T + bo

Sharding: core i <- batch item i (data parallel). The 512MB rel_bias sum is
sharded by query slice: core i reduces rel_bias[0, 64*i:64*(i+1), :, :] over
d; [k,q] pieces are AllGathered.

Stream design: the host supplies the bias slice d-major ([q, d, k], bf16 by
default) so each q is one flat 512KB DMA with 4KB-contiguous lines per
partition (near-peak DMA efficiency; the [q,k,d] layout caps lines at 1KB).
The d-reduction runs on the TENSOR engine: stage tile [128p, 4jj, 512k]
holds d = 4p+jj, and four accumulating matmuls with a ones-column land
bias[q, :] in PSUM row q (exact fp32 accumulation). A small PE-transpose hop
converts piece [64q, 512k] -> [128k, 64q] per k-chunk for the AllGather.
This keeps DVE/ACT free for rope/exp/softmax so they overlap the stream.

exp(s + b) = exp(s) * exp(b): exp(scores) for all heads is computed while
the bias stream is still running; only the elementwise multiply, ctx
matmuls and output projection wait for the AllGather.

All internal layouts are "transposed" (contraction dim on partitions):
    xT [d, l], W?T [din, dout], Q'T/K'T [d, l], scoresT/E [lk, lq],
    ctxT [dh(+1), lq].  Softmax normalization is folded into ctxT via an
    appended ones-column in V (rowsum lands on partition 64) and a
    PE-broadcast reciprocal. The 1/sqrt(dh) scale rides the exp's free
    affine (scale=0.125).
"""
import os
import numpy as np

B, L, D, H = 8, 512, 512, 8
DH = D // H          # 64
NCORES = 8
QS = L // NCORES     # 64 q rows per core
NCH = D // 128       # 4 partition chunks

_cached = {}


def _f32(x):
    return np.ascontiguousarray(x, dtype=np.float32)


def _rope_tables():
    # matches reference _apply_rope: freqs = 10000**(-(arange(0,dh,2)/dh))
    freqs = (10000.0 ** (-(np.arange(0, DH, 2, dtype=np.float32) / np.float32(DH)))).astype(np.float32)
    pos = np.arange(L, dtype=np.float32)
    ang = pos[:, None] * freqs[None, :]          # [L, 32] fp32
    cos = np.cos(ang).astype(np.float32)
    sin = np.sin(ang).astype(np.float32)
    return _f32(np.tile(cos, (1, H))), _f32(np.tile(sin, (1, H)))   # [L, 256]


def _rb_dt():
    return os.environ.get("MHA_RB_DT", "fp8")  # bf16 | fp32 | fp8


FP8_SCALE = 16.0


def _build_nc():
    import concourse.bass as bass
    import concourse.mybir as mybir
    import concourse.tile as tile
    from concourse import bacc

    FP = mybir.dt.float32
    BF = mybir.dt.bfloat16
    AF = mybir.ActivationFunctionType
    ALU = mybir.AluOpType
    rb_dt = _rb_dt()
    RBDT = {"bf16": BF, "fp32": FP, "fp8": mybir.dt.float8e4}[rb_dt]
    STDT = BF if rb_dt == "fp8" else RBDT      # stage tile dtype (fp8 casts on DMA)
    bf16_attn = os.environ.get("MHA_BF16_ATTN", "1") == "1"
    EDT = BF if bf16_attn else FP              # es / eb / e_t / va dtype
    # fp8 streams on gpsimd (cast-DMA), so weights go to HWDGE instead
    wdma_gps = (os.environ.get("MHA_WDMA_GPS", "1") == "1") and rb_dt != "fp8"
    q_pre = int(os.environ.get("MHA_QPRE", "12"))   # q-stages emitted before phase 1a
    stage_bufs = int(os.environ.get("MHA_STAGE_BUFS", "10"))
    dma_split = os.environ.get("MHA_DMA_SPLIT", "1") == "1"
    skip_stream = os.environ.get("MHA_SKIP_STREAM", "0") == "1"
    n_repeat = int(os.environ.get("MHA_REPEAT", "1"))

    nc = bacc.Bacc(None, target_bir_lowering=False, num_devices=NCORES)
    pdma = lambda: (nc.gpsimd if wdma_gps else nc.sync)

    xT_d = nc.dram_tensor("xT", [D, L], FP, kind="ExternalInput")
    rb_d = nc.dram_tensor("rb", [QS, D, L], RBDT, kind="ExternalInput")  # d-major!
    w_d = {n: nc.dram_tensor(n, [D, D], FP, kind="ExternalInput")
           for n in ("wqT", "wkT", "wvT", "woT")}
    b_d = {n: nc.dram_tensor(n, [1, D], FP, kind="ExternalInput")
           for n in ("bq", "bk", "bv", "bo")}
    cos_d = nc.dram_tensor("cost", [L, 256], FP, kind="ExternalInput")
    sin_d = nc.dram_tensor("sint", [L, 256], FP, kind="ExternalInput")
    id_d = nc.dram_tensor("ident", [128, 128], FP, kind="ExternalInput")
    out_d = nc.dram_tensor("out", [L, D], FP, kind="ExternalOutput")
    piece_d = nc.dram_tensor("piece", [128, NCH * QS], EDT)
    gath_d = nc.dram_tensor("gath", [NCORES * 128, NCH * QS], EDT, addr_space="Shared")

    with tile.TileContext(nc) as tc:
        with tc.tile_pool(name="persist", bufs=1) as pp, \
             tc.tile_pool(name="stage", bufs=stage_bufs) as sp:

            # ---------- persistent tiles (SWDGE so HWDGE rings stay free
            # for the bias stream) ----------
            # DMA order on the SWDGE queue = availability order: the
            # projection chain needs xt+wq+rope tables first.
            xt = pp.tile([128, NCH, L], FP)
            pdma().dma_start(out=xt, in_=xT_d.rearrange("(c p) l -> p c l", p=128))
            wts = {}
            for nm in ("wqT",):
                t = pp.tile([128, NCH, D], FP, tag=nm)
                pdma().dma_start(out=t, in_=w_d[nm].rearrange("(c p) j -> p c j", p=128))
                wts[nm] = t
            cost = pp.tile([128, NCH, 256], FP)
            pdma().dma_start(out=cost, in_=cos_d.rearrange("(c p) k -> p c k", p=128))
            sint = pp.tile([128, NCH, 256], FP)
            pdma().dma_start(out=sint, in_=sin_d.rearrange("(c p) k -> p c k", p=128))
            bt = {}
            for nm in ("bq", "bk", "bv", "bo"):
                t = pp.tile([1, D], FP, tag=f"b_{nm}")
                pdma().dma_start(out=t, in_=b_d[nm][:, :])
                bt[nm] = t
            for nm in ("wkT", "wvT"):
                t = pp.tile([128, NCH, D], FP, tag=nm)
                pdma().dma_start(out=t, in_=w_d[nm].rearrange("(c p) j -> p c j", p=128))
                wts[nm] = t
            # wo lives in BOTH partition halves so that head pairs stacked in
            # one PSUM bank (ctx rows 0:64 / 64:128) can both contract with it
            wo_t = pp.tile([128, H, D], FP)
            pdma().dma_start(out=wo_t[0:DH], in_=w_d["woT"].rearrange("(h p) j -> p h j", p=DH))
            pdma().dma_start(out=wo_t[DH:128], in_=w_d["woT"].rearrange("(h p) j -> p h j", p=DH))
            ident = pp.tile([128, 128], FP)
            pdma().dma_start(out=ident, in_=id_d[:, :])
            ones = pp.tile([128, 128], FP)
            nc.vector.memset(ones, 1.0)
            # one-hot column matrix: col 63 is all-ones, rest zero.  The slice
            # colones[:, 63-q : 127-q] is a [128, 64] weight whose only ones-
            # column sits at index q -> PE reduction lands in PSUM row q.
            colones = pp.tile([128, 2 * QS - 1], STDT, tag="colones")
            nc.gpsimd.memset(colones, 0.0)
            nc.gpsimd.memset(colones[:, QS - 1:QS], 1.0)

            va = pp.tile([128, NCH, H * DH], EDT)           # V [lk, (h dh)]
            # column-selector (EDT) for denominator matmuls: col 65 all-ones;
            # slices [65:130] / [1:66] put the ones column at local index
            # 0 / 64 of a [128, 65] weight -> denom lands in PSUM row 0 / 64.
            dsel = pp.tile([128, 130], EDT, tag="dsel")
            nc.gpsimd.memset(dsel, 0.0)
            nc.gpsimd.memset(dsel[:, 65:66], 1.0)
            qt = pp.tile([128, NCH, L], FP)                 # Q'T [d, l]
            kt = pp.tile([128, NCH, L], FP)                 # K'T [d, l]
            eb_k = [pp.tile([128, L], EDT, tag=f"eb{kc}", name=f"eb{kc}")
                    for kc in range(NCH)]                   # exp(biasT) per k-chunk
            pieces = pp.tile([128, NCH, QS], EDT, tag="pieces")  # biasT piece per kc

            def emit_front(esp, ppp, psb):
                    """Streams + phase 1a + phase 1b + piece hop.  Returns es.
                    Runs with ps_piece/piece_sb pools open; caller closes them
                    before phase 2 (which needs all 8 PSUM banks)."""
                    ppsum = ppp.tile([QS, L], FP, tag="ppsum")   # bias[q, k] rows

                    NQG = 1

                    def stream_q(q):
                        st = sp.tile([128, 4, L], STDT, tag="stage")
                        if rb_dt == "fp8":
                            dma_eng = nc.gpsimd      # SWDGE: cast fp8 -> bf16
                        else:
                            dma_eng = nc.scalar if (dma_split and q % 2 == 1) else nc.sync
                        dma_eng.dma_start(
                            out=st, in_=rb_d[q].rearrange("(p four) k -> p four k", four=4))
                        for jj in range(4):
                            nc.tensor.matmul(ppsum, lhsT=colones[:, QS - 1 - q:2 * QS - 1 - q],
                                             rhs=st[:, jj, :],
                                             start=(q == 0 and jj == 0),
                                             stop=(q == QS - 1 and jj == 3),
                                             skip_group_check=True)

                    def piece_hop():
                        """PSUM [64q, 512k] -> pieces [128k, kc, 64q] via PE."""
                        pc = psb.tile([QS, L], FP, tag="piece_sb")
                        nc.scalar.copy(out=pc, in_=ppsum)
                        with tc.tile_pool(name="ps_pt", bufs=2, space="PSUM") as ptp:
                            for kc in range(NCH):
                                tps = ptp.tile([128, QS], FP, tag="pt")
                                nc.tensor.transpose(
                                    tps, in_=pc[:, kc * 128:(kc + 1) * 128],
                                    identity=ident[0:QS, 0:QS])
                                nc.scalar.copy(out=pieces[:, kc, :], in_=tps)

                    # pump(): emit the next n stream q-stages.  Interleaved
                    # through phase 1a/1b so the PE program order alternates
                    # reduce-MM bursts with phase matmuls — otherwise the
                    # stage pool fills and the stream DMA stalls for the
                    # whole phase-1 window.
                    q_it = iter(range(0, QS, NQG))

                    def pump(n):
                        if skip_stream:
                            return
                        for _ in range(n):
                            q = next(q_it, None)
                            if q is None:
                                return
                            stream_q(q)

                    skip_phases = os.environ.get("MHA_SKIP_PHASES", "0") == "1"
                    if skip_phases:
                        for q in range(0, QS, NQG):
                            stream_q(q)
                        piece_hop()
                        nc.gpsimd.dma_start(out=piece_d[:, :],
                                            in_=pieces.rearrange("p a b -> p (a b)"))
                        nc.gpsimd.dma_start(out=out_d[0:128, 0:QS], in_=pieces[:, 0, :])
                        return

                    if skip_stream:
                        nc.vector.memset(pieces.rearrange("p a b -> p (a b)"), 0.01)
                    else:
                        pump(q_pre)

                    # ---------- phase 1a: projections + rope + transposes ----------
                    with tc.tile_pool(name="rope", bufs=1) as rp, \
                         tc.tile_pool(name="ps_a", bufs=3, space="PSUM") as ps_a, \
                         tc.tile_pool(name="ps_tr", bufs=3, space="PSUM") as ps_tr, \
                         tc.tile_pool(name="tmp", bufs=6) as tp:

                        qp = rp.tile([128, NCH, D], FP, tag="qp")   # roped Q [l, d]
                        kp = rp.tile([128, NCH, D], FP, tag="kp")

                        def proj_chunk(wtile, brow, lc):
                            """psum <- x[lc*128:...,:] @ W.T + b  (chunk of 128 l-rows)"""
                            ps = ps_a.tile([128, 512], FP, tag="proj")
                            for kk in range(NCH):
                                nc.tensor.matmul(
                                    ps, lhsT=xt[:, kk, lc * 128:(lc + 1) * 128],
                                    rhs=wtile[:, kk, :],
                                    start=(kk == 0), stop=False)
                            nc.tensor.matmul(ps, lhsT=ones[0:1, 0:128], rhs=brow,
                                             start=False, stop=True)
                            return ps

                        def rope(ps, dst, lc):
                            E = ps.rearrange("p (c two) -> p c two", two=2)[:, :, 0]
                            O = ps.rearrange("p (c two) -> p c two", two=2)[:, :, 1]
                            cc = cost[:, lc, :]
                            ss = sint[:, lc, :]
                            t1 = tp.tile([128, 256], FP, tag="t1")
                            t2 = tp.tile([128, 256], FP, tag="t2")
                            nc.vector.tensor_mul(t1, E, cc)
                            nc.vector.tensor_mul(t2, O, ss)
                            dv = dst[:, lc].rearrange("p (h two k) -> p h two k", two=2, k=32)
                            t1r = t1.rearrange("p (h k) -> p h k", k=32)
                            t2r = t2.rearrange("p (h k) -> p h k", k=32)
                            nc.vector.tensor_sub(dv[:, :, 0, :], t1r, t2r)
                            t3 = tp.tile([128, 256], FP, tag="t1")
                            t4 = tp.tile([128, 256], FP, tag="t2")
                            nc.vector.tensor_mul(t3, E, ss)
                            nc.vector.tensor_mul(t4, O, cc)
                            nc.vector.tensor_add(dv[:, :, 1, :], t3.rearrange("p (h k) -> p h k", k=32),
                                                 t4.rearrange("p (h k) -> p h k", k=32))

                        for lc in range(NCH):
                            ps = proj_chunk(wts["wqT"], bt["bq"], lc)
                            rope(ps, qp, lc)
                            pump(2)
                        for lc in range(NCH):
                            ps = proj_chunk(wts["wkT"], bt["bk"], lc)
                            rope(ps, kp, lc)
                            pump(2)
                        for lc in range(NCH):
                            ps = proj_chunk(wts["wvT"], bt["bv"], lc)
                            nc.scalar.copy(out=va[:, lc], in_=ps)
                            pump(1)

                        # transpose roped Q,K -> [d, l] layout
                        for src, dst in ((qp, qt), (kp, kt)):
                            for lc in range(NCH):
                                for dc in range(NCH):
                                    tps = ps_tr.tile([128, 128], FP, tag="tr")
                                    nc.tensor.transpose(
                                        tps, in_=src[:, lc, dc * 128:(dc + 1) * 128],
                                        identity=ident)
                                    nc.scalar.copy(out=dst[:, dc, lc * 128:(lc + 1) * 128],
                                                   in_=tps)
                                pump(1)

                    # ---------- phase 1b: scores + exp for all heads ----------
                    es = esp.tile([128, H * NCH, L], EDT)    # exp(scoresT/8)
                    with tc.tile_pool(name="ps_s", bufs=3, space="PSUM") as ps_s:
                        for h in range(H):
                            dc, po = h // 2, (h % 2) * DH
                            for m in range(NCH):
                                ps = ps_s.tile([128, 512], FP, tag="sc")
                                nc.tensor.matmul(
                                    ps,
                                    lhsT=kt[po:po + DH, dc, m * 128:(m + 1) * 128],
                                    rhs=qt[po:po + DH, dc, :],
                                    start=True, stop=True)
                                nc.scalar.activation(out=es[:, h * NCH + m, :], in_=ps,
                                                     func=AF.Exp, scale=0.125)
                            pump(2)

                    # ---------- rest of the bias stream ----------
                    if not skip_stream:
                        pump(QS)
                        piece_hop()
                    return es

            def emit_back(es):
                # ---------- collectives: allgather bias pieces per k-chunk.
                # Breadth-first emission so the 4 collectives pipeline on the
                # gpsimd queue instead of serializing behind each other's
                # completion waits. ----------
                eb_scale = (1.0 / FP8_SCALE) if rb_dt == "fp8" else 1.0
                skip_gather = os.environ.get("MHA_SKIP_GATHER", "0") == "1"
                with tc.tile_pool(name="btkp", bufs=1) as btp:
                    if skip_gather:
                        for kc in range(NCH):
                            nc.gpsimd.memset(eb_k[kc], 1.0)
                    else:
                        nc.gpsimd.dma_start(out=piece_d[:, :],
                                            in_=pieces.rearrange("p a b -> p (a b)"))
                        nc.gpsimd.collective_compute(
                            "AllGather", ALU.bypass,
                            replica_groups=[list(range(NCORES))],
                            ins=[piece_d[:, :]], outs=[gath_d[:, :]])
                        btk = btp.tile([128, NCH, NCORES, QS], EDT, tag="btk")
                        nc.gpsimd.dma_start(
                            out=btk,
                            in_=gath_d.rearrange("(j p) (c q) -> p c j q", p=128, c=NCH))
                        for kc in range(NCH):
                            nc.scalar.activation(out=eb_k[kc],
                                                 in_=btk[:, kc].rearrange("p a b -> p (a b)"),
                                                 func=AF.Exp, scale=eb_scale)

                if os.environ.get("MHA_SKIP_P2", "0") == "1":
                    nc.gpsimd.dma_start(out=out_d[0:128, 0:QS], in_=pieces[:, 0, :])
                    return

                # ---------- phase 2: batched + pipelined per head pair to
                # minimize cross-engine dependency round-trips (per-head
                # serial chains cost ~150us in wall latency; the work is
                # ~45us).  Two heads share each ctx PSUM bank (rows 0:64 /
                # 64:128); each pair's softmax denominators land in rows
                # 0/64 of a per-pair PSUM bank via one-hot-selector matmuls,
                # get copied out + exp(-ln(x))-reciprocated immediately
                # (legal ACT bases), and broadcast via ones-row matmuls into
                # separate base-0 banks for the even/odd normalize muls. ----------
                NP = H // 2                                  # head pairs
                with tc.tile_pool(name="emul", bufs=6) as ep, \
                     tc.tile_pool(name="cu", bufs=1) as cup, \
                     tc.tile_pool(name="nrm1", bufs=1) as nr1, \
                     tc.tile_pool(name="outp", bufs=2) as op_, \
                     tc.tile_pool(name="ps_out", bufs=1, space="PSUM") as pout:

                    cu = cup.tile([128, NP, 512], FP, tag="cu")   # ctx pairs
                    cu2 = nr1.tile([DH, NP, 512], FP, tag="cu2")  # odd halves @0
                    brd = nr1.tile([DH + 1, NP, 512], FP, tag="brd")  # recips
                    ops_tiles = [pout.tile([128, 512], FP, tag=f"ops{m}",
                                           name=f"ops{m}")
                                 for m in range(NCH)]
                    with tc.tile_pool(name="ps_den", bufs=2, space="PSUM") as pden, \
                         tc.tile_pool(name="ps_ctx", bufs=2, space="PSUM") as pctx:
                        for pr in range(NP):
                            cps = pctx.tile([128, 512], FP, tag="ctx")
                            dpsp = pden.tile([DH + 1, 512], FP, tag="dps")
                            for hh in range(2):
                                h = 2 * pr + hh
                                for kc in range(NCH):
                                    e_t = ep.tile([128, 512], EDT, tag="e")
                                    nc.vector.tensor_mul(e_t, es[:, h * NCH + kc, :],
                                                         eb_k[kc][:, :])
                                    nc.tensor.matmul(
                                        cps[hh * DH:(hh + 1) * DH, :],
                                        lhsT=va[:, kc, h * DH:(h + 1) * DH],
                                        rhs=e_t,
                                        start=(kc == 0), stop=(kc == NCH - 1),
                                        skip_group_check=True)
                                    # denom of head h -> dpsp row 0 (even) / 64 (odd)
                                    nc.tensor.matmul(
                                        dpsp, lhsT=dsel[:, 65 - hh * DH:130 - hh * DH],
                                        rhs=e_t,
                                        start=(hh == 0 and kc == 0),
                                        stop=(hh == 1 and kc == NCH - 1),
                                        skip_group_check=True)
                            nc.scalar.copy(out=cu[:, pr, :], in_=cps)
                            # per-pair reciprocal rows: 1/x = exp(-ln(x)) on
                            # ACT (denoms positive, O(1e2..1e3); LUT ~1e-5).
                            for base in (0, DH):
                                nc.scalar.copy(out=brd[base:base + 1, pr, :],
                                               in_=dpsp[base:base + 1, :])
                                nc.scalar.activation(out=brd[base:base + 1, pr, :],
                                                     in_=brd[base:base + 1, pr, :],
                                                     func=AF.Ln)
                                nc.scalar.activation(out=brd[base:base + 1, pr, :],
                                                     in_=brd[base:base + 1, pr, :],
                                                     func=AF.Exp, scale=-1.0)
                    # odd-head ctx halves to base 0 (mixed row-group matmuls
                    # inside one PSUM accumulation group hang the PE); this
                    # DMA overlaps the broadcast matmuls below.
                    nc.gpsimd.dma_start(out=cu2, in_=cu[DH:128, :, :])
                    with tc.tile_pool(name="ps_bc", bufs=2, space="PSUM") as pbc:
                        for pr in range(NP):
                            bpsE = pbc.tile([DH, 512], FP, tag="bcE")
                            nc.tensor.matmul(bpsE, lhsT=ones[0:1, 0:DH],
                                             rhs=brd[0:1, pr, :], start=True, stop=True,
                                             skip_group_check=True)
                            nc.vector.tensor_mul(cu[0:DH, pr, :], cu[0:DH, pr, :], bpsE)
                            bpsO = pbc.tile([DH, 512], FP, tag="bcO")
                            nc.tensor.matmul(bpsO, lhsT=ones[DH:DH + 1, 0:DH],
                                             rhs=brd[DH:DH + 1, pr, :],
                                             start=True, stop=True,
                                             skip_group_check=True)
                            nc.vector.tensor_mul(cu2[:, pr, :], cu2[:, pr, :], bpsO)
                        for m in range(NCH):
                            for pr in range(NP):
                                for hh in range(2):
                                    h = 2 * pr + hh
                                    src = cu if hh == 0 else cu2
                                    nc.tensor.matmul(
                                        ops_tiles[m],
                                        lhsT=src[0:DH, pr, m * 128:(m + 1) * 128],
                                        rhs=wo_t[0:DH, h, :],
                                        start=(h == 0), stop=False,
                                        skip_group_check=True)
                            nc.tensor.matmul(ops_tiles[m], lhsT=ones[0:1, 0:128],
                                             rhs=bt["bo"], start=False, stop=True,
                                             skip_group_check=True)
                            osb = op_.tile([128, 512], FP, tag="osb")
                            nc.scalar.copy(out=osb, in_=ops_tiles[m])
                            nc.sync.dma_start(out=out_d[m * 128:(m + 1) * 128, :],
                                              in_=osb)

            def emit_pass():
                with tc.tile_pool(name="es_p", bufs=1) as esp:
                    with tc.tile_pool(name="ps_piece", bufs=1, space="PSUM") as ppp, \
                         tc.tile_pool(name="piece_sb", bufs=1) as psb:
                        es = emit_front(esp, ppp, psb)
                    if es is not None:
                        emit_back(es)

            for _rep in range(n_repeat):
                emit_pass()
    nc.compile()
    return nc


def _in_maps(x, rel_bias, Wq, bq, Wk, bk, Wv, bv, Wo, bo):
    cost, sint = _rope_tables()
    ident = np.eye(128, dtype=np.float32)
    wqT, wkT, wvT, woT = (_f32(np.asarray(W).T) for W in (Wq, Wk, Wv, Wo))
    x = np.asarray(x)
    rel_bias = np.asarray(rel_bias)
    rb_dt = _rb_dt()
    maps = []
    for c in range(NCORES):
        sl = rel_bias[0, c * QS:(c + 1) * QS].transpose(0, 2, 1)  # [q, d, k]
        if rb_dt == "bf16":
            import ml_dtypes
            rbp = np.ascontiguousarray(sl).astype(ml_dtypes.bfloat16)
        elif rb_dt == "fp8":
            import ml_dtypes
            rbp = np.ascontiguousarray(sl * FP8_SCALE).astype(ml_dtypes.float8_e4m3)
        else:
            rbp = _f32(sl)
        maps.append({
            "xT": _f32(x[c].T),
            "rb": rbp,
            "wqT": wqT, "wkT": wkT, "wvT": wvT, "woT": woT,
            "bq": _f32(np.asarray(bq).reshape(1, D)),
            "bk": _f32(np.asarray(bk).reshape(1, D)),
            "bv": _f32(np.asarray(bv).reshape(1, D)),
            "bo": _f32(np.asarray(bo).reshape(1, D)),
            "cost": cost, "sint": sint,
            "ident": ident,
        })
    return maps


def get_nc():
    if "nc" not in _cached:
        _cached["nc"] = _build_nc()
    return _cached["nc"]


def kernel(x, rel_bias, Wq, bq, Wk, bk, Wv, bv, Wo, bo):
    from concourse.bass_utils import run_bass_kernel_spmd
    nc = get_nc()
    maps = _in_maps(x, rel_bias, Wq, bq, Wk, bk, Wv, bv, Wo, bo)
    res = run_bass_kernel_spmd(nc, maps, core_ids=list(range(NCORES)))
    out = np.stack([res.results[c]["out"] for c in range(NCORES)], axis=0)
    return out.astype(np.float32)



# revision 2
# speedup vs baseline: 3.1801x; 3.1801x over previous
"""Trainium2 Bass kernel v2 for MultiHeadAttention with RoPE + summed relative bias.

Reference computation (B=8, L=512, D=512, H=8, dh=64):
    Q,K,V = x @ W{q,k,v}.T + b ; RoPE(Q,K) (concat variant)
    scores = Q K^T / 8 + rel_bias.sum(-1)   (bias broadcast over batch+heads)
    out = softmax(scores) V @ Wo.T + bo

Sharding (v2): core c owns QUERY rows q in [64c, 64c+64) of every batch item
and head.  The bias slice a core needs -- rel_bias.sum(-1)[qslice, :] -- is
exactly the slice it reduces locally from its 16MB fp8 [q, d, k] stream, so
there is NO collective.  K/V (and their projections + rope) are recomputed
per core for all 8 batch items; x is replicated (bf16).

All matmuls run in bf16/fp8 (fp32 PE matmuls cost 4 cycles/row).  The d-
reduction of the bias stream runs on the tensor engine directly from the
fp8 stage tiles (one-hot column selector weights land row q of PSUM), so
the stream is a plain HWDGE fp8 copy with 2KB contiguous lines.

Layouts (contraction dim on partitions):
    xT [d, l] per batch, W?T [din, dout], Q'T/K'T [d, l], scoresT/es [k, q],
    va [k, (kc, h, dh+1)] with a ones column per head (softmax denominator
    lands in PSUM row 64 of the ctx matmul), outT [dout, q] per batch
    (host transposes back).  Softmax normalization: DVE reciprocal of the
    denominator row, PE ones-broadcast, DVE multiply.
"""
import os
import numpy as np

B, L, D, H = 8, 512, 512, 8
DH = D // H          # 64
NCORES = 8
QS = L // NCORES     # 64 q rows per core
NCH = D // 128       # 4 partition chunks

_cached = {}
FP8_SCALE = 16.0


def _bf16(a):
    import ml_dtypes
    return np.ascontiguousarray(np.asarray(a, dtype=np.float32)).astype(ml_dtypes.bfloat16)


def _f32(a):
    return np.ascontiguousarray(a, dtype=np.float32)


def _rope_tables():
    freqs = (10000.0 ** (-(np.arange(0, DH, 2, dtype=np.float32) / np.float32(DH)))).astype(np.float32)
    pos = np.arange(L, dtype=np.float32)
    ang = pos[:, None] * freqs[None, :]          # [L, 32] fp32
    cos = np.cos(ang).astype(np.float32)
    sin = np.sin(ang).astype(np.float32)
    return np.tile(cos, (1, H)), np.tile(sin, (1, H))   # [L, 256]


def _blob_offsets():
    """Element offsets (bf16) of each section within the two packed blobs."""
    off, cur = {}, 0

    def add(name, n):
        nonlocal cur
        off[name] = (cur, cur + n)
        cur += n

    add("xt", B * NCH * L)            # 16384
    for nm in ("wqT", "wkT", "wvT"):
        add(nm, NCH * D)              # 2048 each
    add("woT", H * D)                 # 4096 (rows 0:64)
    add("cost", NCH * 256)            # 1024
    add("sint", NCH * 256)
    add("ident", 128)
    for nm in ("bq", "bk", "bv", "bo"):
        add(f"b_{nm}", D)             # rows 0:1
    off["sh_total"] = cur
    cur = 0
    add("xq", B * NCH * QS)           # 2048
    add("cosq", 256)                  # rows 0:64
    add("sinq", 256)
    off["cb_total"] = cur
    return off


def _build_nc():
    import concourse.bass as bass
    import concourse.mybir as mybir
    import concourse.tile as tile
    from concourse import bacc

    FP = mybir.dt.float32
    BF = mybir.dt.bfloat16
    F8 = mybir.dt.float8e4
    AF = mybir.ActivationFunctionType

    NQG = int(os.environ.get("MHA2_NQG", "4"))      # q rows per stage DMA
    NST = QS // NQG                                 # number of stage tiles
    stage_bufs = int(os.environ.get("MHA2_STAGE_BUFS", "5"))
    q_pre = int(os.environ.get("MHA2_QPRE", "2"))   # stages pre-pumped
    use_dr = os.environ.get("MHA2_DR", "0") == "1"  # DoubleRow reduce
    n_repeat = int(os.environ.get("MHA2_REPEAT", "1"))

    nc = bacc.Bacc(None, target_bir_lowering=False, num_devices=1)

    OFF = _blob_offsets()
    rb_d = nc.dram_tensor("rb", [QS, D, L], F8, kind="ExternalInput")    # [q, d, k]
    sh_d = nc.dram_tensor("shblob", [128, OFF["sh_total"]], BF, kind="ExternalInput")
    cb_d = nc.dram_tensor("cblob", [128, OFF["cb_total"]], BF, kind="ExternalInput")
    out_d = nc.dram_tensor("out", [B, D, QS], BF, kind="ExternalOutput")  # outT per b

    with tile.TileContext(nc) as tc:
        with tc.tile_pool(name="persist", bufs=1) as pp, \
             tc.tile_pool(name="stage", bufs=stage_bufs) as sp:

            # ---------- persistent tiles: two mega-blobs, sliced views ----------
            ones = pp.tile([128, 128], BF)
            nc.vector.memset(ones, 1.0)
            # one-hot selector for the d-reduction: column 63 all-ones.
            # Slice [:, 63-q : 127-q] -> ones-column at local index q.
            # (memsets BEFORE the blob DMAs on the gpsimd queue)
            colones = pp.tile([128, 2 * QS - 1], F8, tag="colones")
            nc.gpsimd.memset(colones, 0.0)
            nc.gpsimd.memset(colones[:, QS - 1:QS], 1.0)
            if use_dr:
                col_dr = pp.tile([128, 2, 2 * QS], F8, tag="col_dr")
                nc.gpsimd.memset(col_dr.rearrange("p a b -> p (a b)"), 0.0)
                nc.gpsimd.memset(col_dr[:, :, QS - 1:QS], 1.0)

            sh = pp.tile([128, OFF["sh_total"]], BF, tag="sh")
            nc.gpsimd.dma_start(out=sh, in_=sh_d[:, :])
            cb = pp.tile([128, OFF["cb_total"]], BF, tag="cb")
            nc.scalar.dma_start(out=cb, in_=cb_d[:, :])

            def sec(name, rows=128):
                a, b_ = OFF[name]
                return sh[0:rows, a:b_]

            xt = sec("xt").rearrange("p (b c l) -> p b c l", b=B, c=NCH)
            wts = {nm: sec(nm).rearrange("p (c j) -> p c j", c=NCH)
                   for nm in ("wqT", "wkT", "wvT")}
            wo_t = sec("woT", DH).rearrange("p (h j) -> p h j", h=H)
            cost = sec("cost").rearrange("p (c k) -> p c k", c=NCH)
            sint = sec("sint").rearrange("p (c k) -> p c k", c=NCH)
            ident = sec("ident")
            bt = {nm: sec(f"b_{nm}", 1) for nm in ("bq", "bk", "bv", "bo")}
            ca, cb_e = OFF["xq"]
            xq = cb[:, ca:cb_e].rearrange("p (b c l) -> p b c l", b=B, c=NCH)
            ca, cb_e = OFF["cosq"]
            cosq = cb[0:QS, ca:cb_e]
            ca, cb_e = OFF["sinq"]
            sinq = cb[0:QS, ca:cb_e]

            es_all = pp.tile([128, B, H, NCH, QS], BF, tag="es_all")  # exp(scoresT/8)
            ebT = pp.tile([128, NCH, QS], BF, tag="ebT")              # exp(biasT)
            qt_all = pp.tile([128, B, NCH, QS], BF, tag="qt")         # Q'T [d, q]
            va_all = pp.tile([128, B, NCH, H, DH + 1], BF, tag="va")  # V [k, ...]+ones

            def emit(first):
                # ---------- bias stream: fp8 [q, d, k] -> PE reduce ----------
                with tc.tile_pool(name="ppsum", bufs=1, space="PSUM") as ppp:
                    ppsum = ppp.tile([QS, L], FP, tag="ppsum")

                    def stream_stage(si):
                        st = sp.tile([128, NQG, 4, L], F8, tag="stage")
                        dma_eng = nc.sync if si % 2 == 0 else nc.scalar
                        dma_eng.dma_start(
                            out=st,
                            in_=rb_d[si * NQG:(si + 1) * NQG].rearrange(
                                "q (p four) k -> p q four k", four=4))
                        for qq in range(NQG):
                            q = si * NQG + qq
                            if use_dr:
                                for jj in range(2):
                                    nc.tensor.matmul(
                                        ppsum,
                                        lhsT=col_dr[:, :, QS - 1 - q:2 * QS - 1 - q],
                                        rhs=st[:, qq, 2 * jj:2 * jj + 2, :],
                                        start=(q == 0 and jj == 0),
                                        stop=(q == QS - 1 and jj == 1),
                                        perf_mode=mybir.MatmulPerfMode.DoubleRow,
                                        skip_group_check=True)
                            else:
                                for jj in range(4):
                                    nc.tensor.matmul(
                                        ppsum,
                                        lhsT=colones[:, QS - 1 - q:2 * QS - 1 - q],
                                        rhs=st[:, qq, jj, :],
                                        start=(q == 0 and jj == 0),
                                        stop=(q == QS - 1 and jj == 3),
                                        skip_group_check=True)

                    st_it = iter(range(NST))

                    def pump(n):
                        for _ in range(n):
                            si = next(st_it, None)
                            if si is None:
                                return
                            stream_stage(si)

                    pump(q_pre)

                    # ---------- phase A: projections + rope + scoresT + es ----------
                    with tc.tile_pool(name="rope", bufs=2) as rp, \
                         tc.tile_pool(name="ktp", bufs=2) as ktp, \
                         tc.tile_pool(name="ps_a", bufs=3, space="PSUM") as ps_a, \
                         tc.tile_pool(name="ps_tr", bufs=2, space="PSUM") as ps_tr, \
                         tc.tile_pool(name="ps_s", bufs=2, space="PSUM") as ps_s, \
                         tc.tile_pool(name="tmp", bufs=4) as tp:

                        def proj(b, wname, bname, qonly):
                            """PSUM [l-rows, 512 dout] for one l-chunk (gen)."""
                            nlc = 1 if qonly else NCH
                            for lc in range(nlc):
                                ps = ps_a.tile([128, D], FP, tag="proj")
                                for kk in range(NCH):
                                    lhsT = (xq[:, b, kk, :] if qonly
                                            else xt[:, b, kk, lc * 128:(lc + 1) * 128])
                                    nc.tensor.matmul(ps[0:QS if qonly else 128, :],
                                                     lhsT=lhsT, rhs=wts[wname][:, kk, :],
                                                     start=(kk == 0), stop=False)
                                nc.tensor.matmul(ps[0:QS if qonly else 128, :],
                                                 lhsT=ones[0:1, 0:QS if qonly else 128],
                                                 rhs=bt[bname], start=False, stop=True)
                                yield lc, ps

                        def rope(ps, nrows, dst, cc, ss):
                            E = ps.rearrange("p (c two) -> p c two", two=2)[0:nrows, :, 0]
                            O = ps.rearrange("p (c two) -> p c two", two=2)[0:nrows, :, 1]
                            t1 = tp.tile([128, 256], FP, tag="t1")
                            t2 = tp.tile([128, 256], FP, tag="t2")
                            nc.vector.tensor_mul(t1[0:nrows], E, cc)
                            nc.vector.tensor_mul(t2[0:nrows], O, ss)
                            dv = dst.rearrange("p (h two k) -> p h two k", two=2, k=32)
                            t1r = t1[0:nrows].rearrange("p (h k) -> p h k", k=32)
                            t2r = t2[0:nrows].rearrange("p (h k) -> p h k", k=32)
                            nc.vector.tensor_sub(dv[:, :, 0, :], t1r, t2r)
                            t3 = tp.tile([128, 256], FP, tag="t1")
                            t4 = tp.tile([128, 256], FP, tag="t2")
                            nc.vector.tensor_mul(t3[0:nrows], E, ss)
                            nc.vector.tensor_mul(t4[0:nrows], O, cc)
                            nc.vector.tensor_add(dv[:, :, 1, :],
                                                 t3[0:nrows].rearrange("p (h k) -> p h k", k=32),
                                                 t4[0:nrows].rearrange("p (h k) -> p h k", k=32))

                        for b in range(B):
                            # --- Q: proj + rope + transpose -> qt_all[:, b] ---
                            qp = rp.tile([QS, D], BF, tag="qp")
                            for _, ps in proj(b, "wqT", "bq", True):
                                rope(ps, QS, qp, cosq, sinq)
                            for dc in range(NCH):
                                tps = ps_tr.tile([128, 128], BF, tag="tr")
                                nc.tensor.transpose(
                                    tps[:, 0:QS], in_=qp[:, dc * 128:(dc + 1) * 128],
                                    identity=ident[0:QS, 0:QS])
                                nc.scalar.copy(out=qt_all[:, b, dc, :], in_=tps[:, 0:QS])
                            pump(1)

                            # --- K: proj + rope + transpose -> kt ---
                            kp = rp.tile([128, NCH, D], BF, tag="kp")
                            for lc, ps in proj(b, "wkT", "bk", False):
                                rope(ps, 128, kp[:, lc], cost[:, lc, :], sint[:, lc, :])
                            kt = ktp.tile([128, NCH, L], BF, tag="kt")
                            for lc in range(NCH):
                                for dc in range(NCH):
                                    tps = ps_tr.tile([128, 128], BF, tag="tr")
                                    nc.tensor.transpose(
                                        tps, in_=kp[:, lc, dc * 128:(dc + 1) * 128],
                                        identity=ident)
                                    nc.vector.tensor_copy(
                                        out=kt[:, dc, lc * 128:(lc + 1) * 128], in_=tps)
                                pump(1 if lc % 2 == 0 else 0)

                            # --- V: proj -> va_all[:, b] (+ ones column) ---
                            nc.vector.memset(va_all[:, b, :, :, DH:DH + 1], 1.0)
                            for lc, ps in proj(b, "wvT", "bv", False):
                                nc.scalar.copy(
                                    out=va_all[:, b, lc, :, 0:DH],
                                    in_=ps.rearrange("p (h d) -> p h d", d=DH))
                            pump(1)

                            # --- scoresT + es for all heads of batch b ---
                            for h in range(H):
                                dc, po = h // 2, (h % 2) * DH
                                sps = ps_s.tile([128, NCH, QS], FP, tag="sc")
                                for m in range(NCH):
                                    nc.tensor.matmul(
                                        sps[:, m, :],
                                        lhsT=kt[po:po + DH, dc, m * 128:(m + 1) * 128],
                                        rhs=qt_all[po:po + DH, b, dc, :],
                                        start=True, stop=True,
                                        skip_group_check=True)
                                nc.scalar.activation(out=es_all[:, b, h], in_=sps,
                                                     func=AF.Exp, scale=0.125)
                            pump(1)

                        pump(NST)

                    # ---------- biasT hop: ppsum [q, k] -> ebT [k, q], exp ----------
                    with tc.tile_pool(name="hop", bufs=1) as hp, \
                         tc.tile_pool(name="ps_h", bufs=2, space="PSUM") as ph:
                        pc = hp.tile([QS, L], FP, tag="pc")
                        nc.scalar.copy(out=pc, in_=ppsum)
                        pcb = hp.tile([QS, L], BF, tag="pcb")
                        nc.vector.tensor_copy(out=pcb, in_=pc)
                        for kc in range(NCH):
                            tps = ph.tile([128, QS], BF, tag="hopt")
                            nc.tensor.transpose(
                                tps, in_=pcb[:, kc * 128:(kc + 1) * 128],
                                identity=ident[0:QS, 0:QS])
                            nc.scalar.activation(out=ebT[:, kc, :], in_=tps,
                                                 func=AF.Exp, scale=1.0 / FP8_SCALE)

                # ---------- phase B: e_t, ctx+den, normalize, out-proj ----------
                with tc.tile_pool(name="emul", bufs=4) as ep, \
                     tc.tile_pool(name="cup", bufs=2) as cup, \
                     tc.tile_pool(name="denp", bufs=2) as dp, \
                     tc.tile_pool(name="outp", bufs=2) as op_, \
                     tc.tile_pool(name="ps_ctx", bufs=4, space="PSUM") as pctx, \
                     tc.tile_pool(name="ps_bc", bufs=2, space="PSUM") as pbc, \
                     tc.tile_pool(name="ps_out", bufs=2, space="PSUM") as pout:
                    for b in range(B):
                        cu = cup.tile([DH, H, QS], BF, tag="cu")
                        den = dp.tile([1, H, QS], FP, tag="den")
                        rec = dp.tile([1, H, QS], BF, tag="rec")
                        for h in range(H):
                            e_t = ep.tile([128, NCH, QS], BF, tag="e")
                            nc.vector.tensor_mul(e_t, es_all[:, b, h], ebT)
                            cps = pctx.tile([DH + 1, QS], FP, tag="ctx")
                            for kc in range(NCH):
                                nc.tensor.matmul(
                                    cps, lhsT=va_all[:, b, kc, h, :],
                                    rhs=e_t[:, kc, :],
                                    start=(kc == 0), stop=(kc == NCH - 1),
                                    skip_group_check=True)
                            nc.scalar.copy(out=den[0:1, h, :], in_=cps[DH:DH + 1, :])
                            nc.scalar.copy(out=cu[:, h, :], in_=cps[0:DH, :])
                        nc.vector.reciprocal(den, den)
                        nc.vector.tensor_copy(out=rec, in_=den)
                        for h in range(H):
                            bps = pbc.tile([DH, QS], FP, tag="bc")
                            nc.tensor.matmul(bps, lhsT=ones[0:1, 0:DH],
                                             rhs=rec[0:1, h, :], start=True, stop=True,
                                             skip_group_check=True)
                            nc.vector.tensor_mul(cu[:, h, :], cu[:, h, :], bps)
                        ops = pout.tile([128, NCH, QS], FP, tag="ops")
                        for ch in range(NCH):
                            for h in range(H):
                                nc.tensor.matmul(
                                    ops[:, ch, :],
                                    lhsT=wo_t[:, h, ch * 128:(ch + 1) * 128],
                                    rhs=cu[:, h, :],
                                    start=(h == 0), stop=False,
                                    skip_group_check=True)
                            nc.tensor.matmul(
                                ops[:, ch, :],
                                lhsT=bt["bo"][0:1, ch * 128:(ch + 1) * 128],
                                rhs=ones[0:1, 0:QS], start=False, stop=True,
                                skip_group_check=True)
                        osb = op_.tile([128, NCH, QS], BF, tag="osb")
                        nc.scalar.copy(out=osb, in_=ops)
                        nc.sync.dma_start(
                            out=out_d[b].rearrange("(c p) q -> p c q", p=128), in_=osb)

            for _rep in range(n_repeat):
                emit(_rep == 0)
    nc.compile()
    return nc


def _pmajor(a, nch=NCH):
    """[R*128? ...] -> p-major [128, rest] image for a [(nch p), cols] tensor."""
    r, cols = a.shape
    assert r == nch * 128
    return a.reshape(nch, 128, cols).transpose(1, 0, 2).reshape(128, nch * cols)


def _in_maps(x, rel_bias, Wq, bq, Wk, bk, Wv, bv, Wo, bo):
    import ml_dtypes
    OFF = _blob_offsets()
    cos, sin = _rope_tables()
    x = np.asarray(x)
    rel_bias = np.asarray(rel_bias)

    sh = np.zeros((128, OFF["sh_total"]), dtype=np.float32)

    def put(name, img, rows=128):
        a, b_ = OFF[name]
        sh[0:rows, a:b_] = img

    xT = _f32(x.transpose(0, 2, 1))                          # [B, D, L]
    # xt section: [p, (b c l)] with d = c*128 + p
    put("xt", xT.reshape(B, NCH, 128, L).transpose(2, 0, 1, 3).reshape(128, -1))
    for nm, W in (("wqT", Wq), ("wkT", Wk), ("wvT", Wv)):
        put(nm, _pmajor(_f32(np.asarray(W).T)))
    # woT rows (h p): p-major over dh=64
    woT = _f32(np.asarray(Wo).T)
    put("woT", woT.reshape(H, DH, D).transpose(1, 0, 2).reshape(DH, H * D), DH)
    put("cost", _pmajor(_f32(cos)))
    put("sint", _pmajor(_f32(sin)))
    put("ident", np.eye(128, dtype=np.float32))
    for nm, b_ in (("bq", bq), ("bk", bk), ("bv", bv), ("bo", bo)):
        put(f"b_{nm}", _f32(np.asarray(b_)).reshape(1, D), 1)
    sh_bf = sh.astype(ml_dtypes.bfloat16)

    maps = []
    for c in range(NCORES):
        sl = rel_bias[0, c * QS:(c + 1) * QS].transpose(0, 2, 1)  # [q, d, k]
        rbp = np.ascontiguousarray(sl * FP8_SCALE).astype(ml_dtypes.float8_e4m3)
        cb = np.zeros((128, OFF["cb_total"]), dtype=np.float32)
        a, b_ = OFF["xq"]
        xqT = _f32(x[:, c * QS:(c + 1) * QS, :].transpose(0, 2, 1))  # [B, D, QS]
        cb[:, a:b_] = xqT.reshape(B, NCH, 128, QS).transpose(2, 0, 1, 3).reshape(128, -1)
        a, b_ = OFF["cosq"]
        cb[0:QS, a:b_] = cos[c * QS:(c + 1) * QS]
        a, b_ = OFF["sinq"]
        cb[0:QS, a:b_] = sin[c * QS:(c + 1) * QS]
        maps.append({"rb": rbp, "shblob": sh_bf,
                     "cblob": cb.astype(ml_dtypes.bfloat16)})
    return maps


def get_nc():
    if "nc" not in _cached:
        _cached["nc"] = _build_nc()
    return _cached["nc"]


def kernel(x, rel_bias, Wq, bq, Wk, bk, Wv, bv, Wo, bo):
    from concourse.bass_utils import run_bass_kernel_spmd
    nc = get_nc()
    maps = _in_maps(x, rel_bias, Wq, bq, Wk, bk, Wv, bv, Wo, bo)
    res = run_bass_kernel_spmd(nc, maps, core_ids=list(range(NCORES)))
    # res[c]["out"]: [B, D, QS] -> full[b, c*QS + q, :] = out[c][b, :, q].T
    out = np.concatenate(
        [np.asarray(res.results[c]["out"], dtype=np.float32).transpose(0, 2, 1)
         for c in range(NCORES)], axis=1)
    return out


# revision 3
# speedup vs baseline: 3.3069x; 1.0399x over previous
"""Trainium2 Bass kernel v2 for MultiHeadAttention with RoPE + summed relative bias.

Reference computation (B=8, L=512, D=512, H=8, dh=64):
    Q,K,V = x @ W{q,k,v}.T + b ; RoPE(Q,K) (concat variant)
    scores = Q K^T / 8 + rel_bias.sum(-1)   (bias broadcast over batch+heads)
    out = softmax(scores) V @ Wo.T + bo

Sharding (v2): core c owns QUERY rows q in [64c, 64c+64) of every batch item
and head.  The bias slice a core needs -- rel_bias.sum(-1)[qslice, :] -- is
exactly the slice it reduces locally from its 16MB fp8 [q, d, k] stream, so
there is NO collective.  K/V (and their projections + rope) are recomputed
per core for all 8 batch items; x is replicated (bf16).

All matmuls run in bf16/fp8 (fp32 PE matmuls cost 4 cycles/row).  The d-
reduction of the bias stream runs on the tensor engine directly from the
fp8 stage tiles (one-hot column selector weights land row q of PSUM), so
the stream is a plain HWDGE fp8 copy with 2KB contiguous lines.

Layouts (contraction dim on partitions):
    xT [d, l] per batch, W?T [din, dout], Q'T/K'T [d, l], scoresT/es [k, q],
    va [k, (kc, h, dh+1)] with a ones column per head (softmax denominator
    lands in PSUM row 64 of the ctx matmul), outT [dout, q] per batch
    (host transposes back).  Softmax normalization: DVE reciprocal of the
    denominator row, PE ones-broadcast, DVE multiply.

Inputs are packed into THREE tensors (rb fp8 + one shared bf16 blob + one
per-core bf16 blob): each ExternalInput binding costs ~56us per exec in this
environment, so 18 tensors -> 3 was worth ~500us/call.  num_devices=1 drops
the partition_id binding (no collectives, SPMD purely via per-core data).
"""
import os
import numpy as np

B, L, D, H = 8, 512, 512, 8
DH = D // H          # 64
NCORES = 8
QS = L // NCORES     # 64 q rows per core
NCH = D // 128       # 4 partition chunks

_cached = {}
FP8_SCALE = 16.0


def _bf16(a):
    import ml_dtypes
    return np.ascontiguousarray(np.asarray(a, dtype=np.float32)).astype(ml_dtypes.bfloat16)


def _f32(a):
    return np.ascontiguousarray(a, dtype=np.float32)


def _rope_tables():
    freqs = (10000.0 ** (-(np.arange(0, DH, 2, dtype=np.float32) / np.float32(DH)))).astype(np.float32)
    pos = np.arange(L, dtype=np.float32)
    ang = pos[:, None] * freqs[None, :]          # [L, 32] fp32
    cos = np.cos(ang).astype(np.float32)
    sin = np.sin(ang).astype(np.float32)
    return np.tile(cos, (1, H)), np.tile(sin, (1, H))   # [L, 256]


def _blob_offsets():
    """Element offsets (bf16) of each section within the two packed blobs."""
    off, cur = {}, 0

    def add(name, n):
        nonlocal cur
        off[name] = (cur, cur + n)
        cur += n

    add("xt", B * NCH * L)            # 16384
    for nm in ("wqT", "wkT", "wvT"):
        add(nm, NCH * D)              # 2048 each
    add("woT", H * D)                 # 4096 (rows 0:64)
    add("cost", NCH * 256)            # 1024
    add("sint", NCH * 256)
    add("ident", 128)
    for nm in ("bq", "bk", "bv", "bo"):
        add(f"b_{nm}", D)             # rows 0:1
    off["sh_total"] = cur
    cur = 0
    add("xq", B * NCH * QS)           # 2048
    add("cosq", 256)                  # rows 0:64
    add("sinq", 256)
    off["cb_total"] = cur
    return off


def _build_nc():
    import concourse.bass as bass
    import concourse.mybir as mybir
    import concourse.tile as tile
    from concourse import bacc

    FP = mybir.dt.float32
    BF = mybir.dt.bfloat16
    F8 = mybir.dt.float8e4
    AF = mybir.ActivationFunctionType

    NQG = int(os.environ.get("MHA2_NQG", "4"))      # q rows per stage DMA
    NST = QS // NQG                                 # number of stage tiles
    stage_bufs = int(os.environ.get("MHA2_STAGE_BUFS", "5"))
    q_pre = int(os.environ.get("MHA2_QPRE", "2"))   # stages pre-pumped
    use_dr = os.environ.get("MHA2_DR", "0") == "1"  # DoubleRow reduce
    n_repeat = int(os.environ.get("MHA2_REPEAT", "1"))

    nc = bacc.Bacc(None, target_bir_lowering=False, num_devices=1)

    OFF = _blob_offsets()
    rb_d = nc.dram_tensor("rb", [QS, D, L], F8, kind="ExternalInput")    # [q, d, k]
    sh_d = nc.dram_tensor("shblob", [128, OFF["sh_total"]], BF, kind="ExternalInput")
    cb_d = nc.dram_tensor("cblob", [128, OFF["cb_total"]], BF, kind="ExternalInput")
    out_d = nc.dram_tensor("out", [B, D, QS], BF, kind="ExternalOutput")  # outT per b

    with tile.TileContext(nc) as tc:
        with tc.tile_pool(name="persist", bufs=1) as pp, \
             tc.tile_pool(name="stage", bufs=stage_bufs) as sp:

            # ---------- persistent tiles: two mega-blobs, sliced views ----------
            ones = pp.tile([128, 128], BF)
            nc.vector.memset(ones, 1.0)
            # one-hot selector for the d-reduction: column 63 all-ones.
            # Slice [:, 63-q : 127-q] -> ones-column at local index q.
            # (memsets BEFORE the blob DMAs on the gpsimd queue)
            colones = pp.tile([128, 2 * QS - 1], F8, tag="colones")
            nc.gpsimd.memset(colones, 0.0)
            nc.gpsimd.memset(colones[:, QS - 1:QS], 1.0)
            if use_dr:
                col_dr = pp.tile([128, 2, 2 * QS], F8, tag="col_dr")
                nc.gpsimd.memset(col_dr.rearrange("p a b -> p (a b)"), 0.0)
                nc.gpsimd.memset(col_dr[:, :, QS - 1:QS], 1.0)

            sh = pp.tile([128, OFF["sh_total"]], BF, tag="sh")
            nc.gpsimd.dma_start(out=sh, in_=sh_d[:, :])
            cb = pp.tile([128, OFF["cb_total"]], BF, tag="cb")
            nc.scalar.dma_start(out=cb, in_=cb_d[:, :])

            def sec(name, rows=128):
                a, b_ = OFF[name]
                return sh[0:rows, a:b_]

            xt = sec("xt").rearrange("p (b c l) -> p b c l", b=B, c=NCH)
            wts = {nm: sec(nm).rearrange("p (c j) -> p c j", c=NCH)
                   for nm in ("wqT", "wkT", "wvT")}
            wo_t = sec("woT", DH).rearrange("p (h j) -> p h j", h=H)
            cost = sec("cost").rearrange("p (c k) -> p c k", c=NCH)
            sint = sec("sint").rearrange("p (c k) -> p c k", c=NCH)
            ident = sec("ident")
            bt = {nm: sec(f"b_{nm}", 1) for nm in ("bq", "bk", "bv", "bo")}
            ca, cb_e = OFF["xq"]
            xq = cb[:, ca:cb_e].rearrange("p (b c l) -> p b c l", b=B, c=NCH)
            ca, cb_e = OFF["cosq"]
            cosq = cb[0:QS, ca:cb_e]
            ca, cb_e = OFF["sinq"]
            sinq = cb[0:QS, ca:cb_e]

            es_all = pp.tile([128, B, H, NCH, QS], BF, tag="es_all")  # exp(scoresT/8)
            ebT = pp.tile([128, NCH, QS], BF, tag="ebT")              # exp(biasT)
            qt_all = pp.tile([128, B, NCH, QS], BF, tag="qt")         # Q'T [d, q]
            va_all = pp.tile([128, B, NCH, H, DH + 1], BF, tag="va")  # V [k, ...]+ones

            def emit(first):
                # ---------- bias stream: fp8 [q, d, k] -> PE reduce ----------
                with tc.tile_pool(name="ppsum", bufs=1, space="PSUM") as ppp:
                    ppsum = ppp.tile([QS, L], FP, tag="ppsum")

                    def stream_stage(si):
                        st = sp.tile([128, NQG, 4, L], F8, tag="stage")
                        dma_eng = nc.sync if si % 2 == 0 else nc.scalar
                        dma_eng.dma_start(
                            out=st,
                            in_=rb_d[si * NQG:(si + 1) * NQG].rearrange(
                                "q (p four) k -> p q four k", four=4))
                        for qq in range(NQG):
                            q = si * NQG + qq
                            if use_dr:
                                for jj in range(2):
                                    nc.tensor.matmul(
                                        ppsum,
                                        lhsT=col_dr[:, :, QS - 1 - q:2 * QS - 1 - q],
                                        rhs=st[:, qq, 2 * jj:2 * jj + 2, :],
                                        start=(q == 0 and jj == 0),
                                        stop=(q == QS - 1 and jj == 1),
                                        perf_mode=mybir.MatmulPerfMode.DoubleRow,
                                        skip_group_check=True)
                            else:
                                for jj in range(4):
                                    nc.tensor.matmul(
                                        ppsum,
                                        lhsT=colones[:, QS - 1 - q:2 * QS - 1 - q],
                                        rhs=st[:, qq, jj, :],
                                        start=(q == 0 and jj == 0),
                                        stop=(q == QS - 1 and jj == 3),
                                        skip_group_check=True)

                    st_it = iter(range(NST))

                    def pump(n):
                        for _ in range(n):
                            si = next(st_it, None)
                            if si is None:
                                return
                            stream_stage(si)

                    pump(q_pre)

                    # ---------- phase A: projections + rope + scoresT + es ----------
                    with tc.tile_pool(name="rope", bufs=2) as rp, \
                         tc.tile_pool(name="ktp", bufs=2) as ktp, \
                         tc.tile_pool(name="ps_a", bufs=3, space="PSUM") as ps_a, \
                         tc.tile_pool(name="ps_tr", bufs=2, space="PSUM") as ps_tr, \
                         tc.tile_pool(name="ps_s", bufs=2, space="PSUM") as ps_s, \
                         tc.tile_pool(name="tmp", bufs=4) as tp:

                        def proj(b, wname, bname, qonly):
                            """PSUM [l-rows, 512 dout] for one l-chunk (gen)."""
                            nlc = 1 if qonly else NCH
                            for lc in range(nlc):
                                ps = ps_a.tile([128, D], FP, tag="proj")
                                for kk in range(NCH):
                                    lhsT = (xq[:, b, kk, :] if qonly
                                            else xt[:, b, kk, lc * 128:(lc + 1) * 128])
                                    nc.tensor.matmul(ps[0:QS if qonly else 128, :],
                                                     lhsT=lhsT, rhs=wts[wname][:, kk, :],
                                                     start=(kk == 0), stop=False)
                                nc.tensor.matmul(ps[0:QS if qonly else 128, :],
                                                 lhsT=ones[0:1, 0:QS if qonly else 128],
                                                 rhs=bt[bname], start=False, stop=True)
                                yield lc, ps

                        def rope(ps, nrows, dst, cc, ss):
                            E = ps.rearrange("p (c two) -> p c two", two=2)[0:nrows, :, 0]
                            O = ps.rearrange("p (c two) -> p c two", two=2)[0:nrows, :, 1]
                            t1 = tp.tile([128, 256], FP, tag="t1")
                            t2 = tp.tile([128, 256], FP, tag="t2")
                            nc.vector.tensor_mul(t1[0:nrows], E, cc)
                            nc.vector.tensor_mul(t2[0:nrows], O, ss)
                            dv = dst.rearrange("p (h two k) -> p h two k", two=2, k=32)
                            t1r = t1[0:nrows].rearrange("p (h k) -> p h k", k=32)
                            t2r = t2[0:nrows].rearrange("p (h k) -> p h k", k=32)
                            nc.vector.tensor_sub(dv[:, :, 0, :], t1r, t2r)
                            t3 = tp.tile([128, 256], FP, tag="t1")
                            t4 = tp.tile([128, 256], FP, tag="t2")
                            nc.vector.tensor_mul(t3[0:nrows], E, ss)
                            nc.vector.tensor_mul(t4[0:nrows], O, cc)
                            nc.vector.tensor_add(dv[:, :, 1, :],
                                                 t3[0:nrows].rearrange("p (h k) -> p h k", k=32),
                                                 t4[0:nrows].rearrange("p (h k) -> p h k", k=32))

                        for b in range(B):
                            # --- Q: proj + rope + transpose -> qt_all[:, b] ---
                            qp = rp.tile([QS, D], BF, tag="qp")
                            for _, ps in proj(b, "wqT", "bq", True):
                                rope(ps, QS, qp, cosq, sinq)
                            for dc in range(NCH):
                                tps = ps_tr.tile([128, 128], BF, tag="tr")
                                nc.tensor.transpose(
                                    tps[:, 0:QS], in_=qp[:, dc * 128:(dc + 1) * 128],
                                    identity=ident[0:QS, 0:QS])
                                nc.scalar.copy(out=qt_all[:, b, dc, :], in_=tps[:, 0:QS])
                            pump(1)

                            # --- K: proj + rope + transpose -> kt ---
                            kp = rp.tile([128, NCH, D], BF, tag="kp")
                            for lc, ps in proj(b, "wkT", "bk", False):
                                rope(ps, 128, kp[:, lc], cost[:, lc, :], sint[:, lc, :])
                            kt = ktp.tile([128, NCH, L], BF, tag="kt")
                            for lc in range(NCH):
                                for dc in range(NCH):
                                    tps = ps_tr.tile([128, 128], BF, tag="tr")
                                    nc.tensor.transpose(
                                        tps, in_=kp[:, lc, dc * 128:(dc + 1) * 128],
                                        identity=ident)
                                    nc.vector.tensor_copy(
                                        out=kt[:, dc, lc * 128:(lc + 1) * 128], in_=tps)
                                pump(1 if lc % 2 == 0 else 0)

                            # --- V: proj -> va_all[:, b] (+ ones column) ---
                            nc.vector.memset(va_all[:, b, :, :, DH:DH + 1], 1.0)
                            for lc, ps in proj(b, "wvT", "bv", False):
                                nc.scalar.copy(
                                    out=va_all[:, b, lc, :, 0:DH],
                                    in_=ps.rearrange("p (h d) -> p h d", d=DH))
                            pump(1)

                            # --- scoresT + es for all heads of batch b ---
                            for h in range(H):
                                dc, po = h // 2, (h % 2) * DH
                                sps = ps_s.tile([128, NCH, QS], FP, tag="sc")
                                for m in range(NCH):
                                    nc.tensor.matmul(
                                        sps[:, m, :],
                                        lhsT=kt[po:po + DH, dc, m * 128:(m + 1) * 128],
                                        rhs=qt_all[po:po + DH, b, dc, :],
                                        start=True, stop=True,
                                        skip_group_check=True)
                                nc.scalar.activation(out=es_all[:, b, h], in_=sps,
                                                     func=AF.Exp, scale=0.125)
                            pump(1)

                        pump(NST)

                    # ---------- biasT hop: ppsum [q, k] -> ebT [k, q], exp ----------
                    with tc.tile_pool(name="hop", bufs=1) as hp, \
                         tc.tile_pool(name="ps_h", bufs=2, space="PSUM") as ph:
                        pc = hp.tile([QS, L], FP, tag="pc")
                        nc.scalar.copy(out=pc, in_=ppsum)
                        pcb = hp.tile([QS, L], BF, tag="pcb")
                        nc.vector.tensor_copy(out=pcb, in_=pc)
                        for kc in range(NCH):
                            tps = ph.tile([128, QS], BF, tag="hopt")
                            nc.tensor.transpose(
                                tps, in_=pcb[:, kc * 128:(kc + 1) * 128],
                                identity=ident[0:QS, 0:QS])
                            nc.scalar.activation(out=ebT[:, kc, :], in_=tps,
                                                 func=AF.Exp, scale=1.0 / FP8_SCALE)

                # ---------- phase B: e_t, ctx+den, normalize, out-proj ----------
                with tc.tile_pool(name="emul", bufs=4) as ep, \
                     tc.tile_pool(name="cup", bufs=2) as cup, \
                     tc.tile_pool(name="denp", bufs=2) as dp, \
                     tc.tile_pool(name="outp", bufs=2) as op_, \
                     tc.tile_pool(name="ps_ctx", bufs=4, space="PSUM") as pctx, \
                     tc.tile_pool(name="ps_bc", bufs=2, space="PSUM") as pbc, \
                     tc.tile_pool(name="ps_out", bufs=2, space="PSUM") as pout:
                    for b in range(B):
                        cu = cup.tile([DH, H, QS], BF, tag="cu")
                        den = dp.tile([1, H, QS], FP, tag="den")
                        rec = dp.tile([1, H, QS], BF, tag="rec")
                        for h in range(H):
                            e_t = ep.tile([128, NCH, QS], BF, tag="e")
                            nc.vector.tensor_mul(e_t, es_all[:, b, h], ebT)
                            cps = pctx.tile([DH + 1, QS], FP, tag="ctx")
                            for kc in range(NCH):
                                nc.tensor.matmul(
                                    cps, lhsT=va_all[:, b, kc, h, :],
                                    rhs=e_t[:, kc, :],
                                    start=(kc == 0), stop=(kc == NCH - 1),
                                    skip_group_check=True)
                            nc.scalar.copy(out=den[0:1, h, :], in_=cps[DH:DH + 1, :])
                            nc.scalar.copy(out=cu[:, h, :], in_=cps[0:DH, :])
                        nc.vector.reciprocal(den, den)
                        nc.vector.tensor_copy(out=rec, in_=den)
                        for h in range(H):
                            bps = pbc.tile([DH, QS], FP, tag="bc")
                            nc.tensor.matmul(bps, lhsT=ones[0:1, 0:DH],
                                             rhs=rec[0:1, h, :], start=True, stop=True,
                                             skip_group_check=True)
                            nc.vector.tensor_mul(cu[:, h, :], cu[:, h, :], bps)
                        ops = pout.tile([128, NCH, QS], FP, tag="ops")
                        for ch in range(NCH):
                            for h in range(H):
                                nc.tensor.matmul(
                                    ops[:, ch, :],
                                    lhsT=wo_t[:, h, ch * 128:(ch + 1) * 128],
                                    rhs=cu[:, h, :],
                                    start=(h == 0), stop=False,
                                    skip_group_check=True)
                            nc.tensor.matmul(
                                ops[:, ch, :],
                                lhsT=bt["bo"][0:1, ch * 128:(ch + 1) * 128],
                                rhs=ones[0:1, 0:QS], start=False, stop=True,
                                skip_group_check=True)
                        osb = op_.tile([128, NCH, QS], BF, tag="osb")
                        nc.scalar.copy(out=osb, in_=ops)
                        nc.sync.dma_start(
                            out=out_d[b].rearrange("(c p) q -> p c q", p=128), in_=osb)

            for _rep in range(n_repeat):
                emit(_rep == 0)
    nc.compile()
    return nc


def _pmajor(a, nch=NCH):
    """[R*128? ...] -> p-major [128, rest] image for a [(nch p), cols] tensor."""
    r, cols = a.shape
    assert r == nch * 128
    return a.reshape(nch, 128, cols).transpose(1, 0, 2).reshape(128, nch * cols)


def _in_maps(x, rel_bias, Wq, bq, Wk, bk, Wv, bv, Wo, bo):
    import ml_dtypes
    OFF = _blob_offsets()
    cos, sin = _rope_tables()
    x = np.asarray(x)
    rel_bias = np.asarray(rel_bias)

    sh = np.zeros((128, OFF["sh_total"]), dtype=np.float32)

    def put(name, img, rows=128):
        a, b_ = OFF[name]
        sh[0:rows, a:b_] = img

    xT = _f32(x.transpose(0, 2, 1))                          # [B, D, L]
    # xt section: [p, (b c l)] with d = c*128 + p
    put("xt", xT.reshape(B, NCH, 128, L).transpose(2, 0, 1, 3).reshape(128, -1))
    for nm, W in (("wqT", Wq), ("wkT", Wk), ("wvT", Wv)):
        put(nm, _pmajor(_f32(np.asarray(W).T)))
    # woT rows (h p): p-major over dh=64
    woT = _f32(np.asarray(Wo).T)
    put("woT", woT.reshape(H, DH, D).transpose(1, 0, 2).reshape(DH, H * D), DH)
    put("cost", _pmajor(_f32(cos)))
    put("sint", _pmajor(_f32(sin)))
    put("ident", np.eye(128, dtype=np.float32))
    for nm, b_ in (("bq", bq), ("bk", bk), ("bv", bv), ("bo", bo)):
        put(f"b_{nm}", _f32(np.asarray(b_)).reshape(1, D), 1)
    sh_bf = sh.astype(ml_dtypes.bfloat16)

    maps = []
    for c in range(NCORES):
        sl = rel_bias[0, c * QS:(c + 1) * QS].transpose(0, 2, 1)  # [q, d, k]
        rbp = np.ascontiguousarray(sl * FP8_SCALE).astype(ml_dtypes.float8_e4m3)
        cb = np.zeros((128, OFF["cb_total"]), dtype=np.float32)
        a, b_ = OFF["xq"]
        xqT = _f32(x[:, c * QS:(c + 1) * QS, :].transpose(0, 2, 1))  # [B, D, QS]
        cb[:, a:b_] = xqT.reshape(B, NCH, 128, QS).transpose(2, 0, 1, 3).reshape(128, -1)
        a, b_ = OFF["cosq"]
        cb[0:QS, a:b_] = cos[c * QS:(c + 1) * QS]
        a, b_ = OFF["sinq"]
        cb[0:QS, a:b_] = sin[c * QS:(c + 1) * QS]
        maps.append({"rb": rbp, "shblob": sh_bf,
                     "cblob": cb.astype(ml_dtypes.bfloat16)})
    return maps


def get_nc():
    if "nc" not in _cached:
        _cached["nc"] = _build_nc()
    return _cached["nc"]


def kernel(x, rel_bias, Wq, bq, Wk, bk, Wv, bv, Wo, bo):
    from concourse.bass_utils import run_bass_kernel_spmd
    nc = get_nc()
    maps = _in_maps(x, rel_bias, Wq, bq, Wk, bk, Wv, bv, Wo, bo)
    res = run_bass_kernel_spmd(nc, maps, core_ids=list(range(NCORES)))
    # res[c]["out"]: [B, D, QS] -> full[b, c*QS + q, :] = out[c][b, :, q].T
    out = np.concatenate(
        [np.asarray(res.results[c]["out"], dtype=np.float32).transpose(0, 2, 1)
         for c in range(NCORES)], axis=1)
    return out


# revision 4
# speedup vs baseline: 3.8623x; 1.1679x over previous
"""Trainium2 Bass kernel v2 for MultiHeadAttention with RoPE + summed relative bias.

Reference computation (B=8, L=512, D=512, H=8, dh=64):
    Q,K,V = x @ W{q,k,v}.T + b ; RoPE(Q,K) (concat variant)
    scores = Q K^T / 8 + rel_bias.sum(-1)   (bias broadcast over batch+heads)
    out = softmax(scores) V @ Wo.T + bo

Sharding (v2): core c owns QUERY rows q in [64c, 64c+64) of every batch item
and head.  The bias slice a core needs -- rel_bias.sum(-1)[qslice, :] -- is
exactly the slice it reduces locally from its 16MB fp8 [q, d, k] stream, so
there is NO collective.  K/V (and their projections + rope) are recomputed
per core for all 8 batch items; x is replicated (bf16).

All matmuls run in bf16/fp8 (fp32 PE matmuls cost 4 cycles/row).  The d-
reduction of the bias stream runs on the tensor engine directly from the
fp8 stage tiles (one-hot column selector weights land row q of PSUM), so
the stream is a plain HWDGE fp8 copy with 2KB contiguous lines.

Layouts (contraction dim on partitions):
    xT [d, l] per batch, W?T [din, dout], Q'T/K'T [d, l], scoresT/es [k, q],
    va [k, (kc, h, dh+1)] with a ones column per head (softmax denominator
    lands in PSUM row 64 of the ctx matmul), outT [dout, q] per batch
    (host transposes back).  Softmax normalization: DVE reciprocal of the
    denominator row, PE ones-broadcast, DVE multiply.
"""
import os
import numpy as np

B, L, D, H = 8, 512, 512, 8
DH = D // H          # 64
NCORES = 8
QS = L // NCORES     # 64 q rows per core
NCH = D // 128       # 4 partition chunks

_cached = {}
FP8_SCALE = 16.0


def _bf16(a):
    import ml_dtypes
    return np.ascontiguousarray(np.asarray(a, dtype=np.float32)).astype(ml_dtypes.bfloat16)


def _f32(a):
    return np.ascontiguousarray(a, dtype=np.float32)


def _rope_tables():
    freqs = (10000.0 ** (-(np.arange(0, DH, 2, dtype=np.float32) / np.float32(DH)))).astype(np.float32)
    pos = np.arange(L, dtype=np.float32)
    ang = pos[:, None] * freqs[None, :]          # [L, 32] fp32
    cos = np.cos(ang).astype(np.float32)
    sin = np.sin(ang).astype(np.float32)
    return np.tile(cos, (1, H)), np.tile(sin, (1, H))   # [L, 256]


def _blob_offsets():
    """Element offsets (bf16) of each section within the two packed blobs."""
    off, cur = {}, 0

    def add(name, n):
        nonlocal cur
        off[name] = (cur, cur + n)
        cur += n

    add("xt", B * NCH * L)            # 16384
    for nm in ("wqT", "wkT", "wvT"):
        add(nm, NCH * D)              # 2048 each
    add("woT", H * D)                 # 4096 (rows 0:64)
    add("cost", NCH * 256)            # 1024
    add("sint", NCH * 256)
    add("ident", 128)
    for nm in ("bq", "bk", "bv", "bo"):
        add(f"b_{nm}", D)             # rows 0:1
    add("xq", B * NCH * QS)           # 2048 (per-core section)
    add("cosq", 256)                  # rows 0:64 (per-core)
    add("sinq", 256)                  # rows 0:64 (per-core)
    off["sh_total"] = cur
    return off


def _build_nc():
    import concourse.bass as bass
    import concourse.mybir as mybir
    import concourse.tile as tile
    from concourse import bacc

    FP = mybir.dt.float32
    BF = mybir.dt.bfloat16
    F8 = mybir.dt.float8e4
    AF = mybir.ActivationFunctionType

    NQG = int(os.environ.get("MHA2_NQG", "4"))      # q rows per stage DMA
    NST = QS // NQG                                 # number of stage tiles
    stage_bufs = int(os.environ.get("MHA2_STAGE_BUFS", "5"))
    q_pre = int(os.environ.get("MHA2_QPRE", "2"))   # stages pre-pumped
    use_dr = os.environ.get("MHA2_DR", "0") == "1"  # DoubleRow reduce
    n_repeat = int(os.environ.get("MHA2_REPEAT", "1"))

    nc = bacc.Bacc(None, target_bir_lowering=False, num_devices=1)

    OFF = _blob_offsets()
    rb_d = nc.dram_tensor("rb", [QS, D, L], F8, kind="ExternalInput")    # [q, d, k]
    sh_d = nc.dram_tensor("shblob", [128, OFF["sh_total"]], BF, kind="ExternalInput")
    out_d = nc.dram_tensor("out", [B, D, QS], BF, kind="ExternalOutput")  # outT per b

    with tile.TileContext(nc) as tc:
        with tc.tile_pool(name="persist", bufs=1) as pp, \
             tc.tile_pool(name="stage", bufs=stage_bufs) as sp:

            # ---------- persistent tiles: two mega-blobs, sliced views ----------
            ones = pp.tile([128, 128], BF)
            nc.vector.memset(ones, 1.0)
            # one-hot selector for the d-reduction: column 63 all-ones.
            # Slice [:, 63-q : 127-q] -> ones-column at local index q.
            # (memsets BEFORE the blob DMAs on the gpsimd queue)
            colones = pp.tile([128, 2 * QS - 1], F8, tag="colones")
            nc.gpsimd.memset(colones, 0.0)
            nc.gpsimd.memset(colones[:, QS - 1:QS], 1.0)
            if use_dr:
                col_dr = pp.tile([128, 2, 2 * QS], F8, tag="col_dr")
                nc.gpsimd.memset(col_dr.rearrange("p a b -> p (a b)"), 0.0)
                nc.gpsimd.memset(col_dr[:, :, QS - 1:QS], 1.0)

            sh = pp.tile([128, OFF["sh_total"]], BF, tag="sh")
            nc.gpsimd.dma_start(out=sh, in_=sh_d[:, :])

            def sec(name, rows=128):
                a, b_ = OFF[name]
                return sh[0:rows, a:b_]

            xt = sec("xt").rearrange("p (b c l) -> p b c l", b=B, c=NCH)
            wts = {nm: sec(nm).rearrange("p (c j) -> p c j", c=NCH)
                   for nm in ("wqT", "wkT", "wvT")}
            wo_t = sec("woT", DH).rearrange("p (h j) -> p h j", h=H)
            cost = sec("cost").rearrange("p (c k) -> p c k", c=NCH)
            sint = sec("sint").rearrange("p (c k) -> p c k", c=NCH)
            ident = sec("ident")
            bt = {nm: sec(f"b_{nm}", 1) for nm in ("bq", "bk", "bv", "bo")}
            xq = sec("xq").rearrange("p (b c l) -> p b c l", b=B, c=NCH)
            cosq = sec("cosq", QS)
            sinq = sec("sinq", QS)

            es_all = pp.tile([128, B, H, NCH, QS], BF, tag="es_all")  # exp(scoresT/8)
            ebT = pp.tile([128, NCH, QS], BF, tag="ebT")              # exp(biasT)
            qt_all = pp.tile([128, B, NCH, QS], BF, tag="qt")         # Q'T [d, q]
            va_all = pp.tile([128, B, NCH, H, DH + 1], BF, tag="va")  # V [k, ...]+ones

            def emit(first):
                # ---------- bias stream: fp8 [q, d, k] -> PE reduce ----------
                with tc.tile_pool(name="ppsum", bufs=1, space="PSUM") as ppp:
                    ppsum = ppp.tile([QS, L], FP, tag="ppsum")

                    def stream_stage(si):
                        st = sp.tile([128, NQG, 4, L], F8, tag="stage")
                        dma_eng = nc.sync if si % 2 == 0 else nc.scalar
                        dma_eng.dma_start(
                            out=st,
                            in_=rb_d[si * NQG:(si + 1) * NQG].rearrange(
                                "q (p four) k -> p q four k", four=4))
                        for qq in range(NQG):
                            q = si * NQG + qq
                            if use_dr:
                                for jj in range(2):
                                    nc.tensor.matmul(
                                        ppsum,
                                        lhsT=col_dr[:, :, QS - 1 - q:2 * QS - 1 - q],
                                        rhs=st[:, qq, 2 * jj:2 * jj + 2, :],
                                        start=(q == 0 and jj == 0),
                                        stop=(q == QS - 1 and jj == 1),
                                        perf_mode=mybir.MatmulPerfMode.DoubleRow,
                                        skip_group_check=True)
                            else:
                                for jj in range(4):
                                    nc.tensor.matmul(
                                        ppsum,
                                        lhsT=colones[:, QS - 1 - q:2 * QS - 1 - q],
                                        rhs=st[:, qq, jj, :],
                                        start=(q == 0 and jj == 0),
                                        stop=(q == QS - 1 and jj == 3),
                                        skip_group_check=True)

                    st_it = iter(range(NST))

                    def pump(n):
                        for _ in range(n):
                            si = next(st_it, None)
                            if si is None:
                                return
                            stream_stage(si)

                    pump(q_pre)

                    # ---------- phase A: projections + rope + scoresT + es ----------
                    with tc.tile_pool(name="rope", bufs=2) as rp, \
                         tc.tile_pool(name="ktp", bufs=2) as ktp, \
                         tc.tile_pool(name="ps_a", bufs=3, space="PSUM") as ps_a, \
                         tc.tile_pool(name="ps_tr", bufs=2, space="PSUM") as ps_tr, \
                         tc.tile_pool(name="ps_s", bufs=2, space="PSUM") as ps_s, \
                         tc.tile_pool(name="tmp", bufs=4) as tp:

                        def proj(b, wname, bname, qonly):
                            """PSUM [l-rows, 512 dout] for one l-chunk (gen)."""
                            nlc = 1 if qonly else NCH
                            for lc in range(nlc):
                                ps = ps_a.tile([128, D], FP, tag="proj")
                                for kk in range(NCH):
                                    lhsT = (xq[:, b, kk, :] if qonly
                                            else xt[:, b, kk, lc * 128:(lc + 1) * 128])
                                    nc.tensor.matmul(ps[0:QS if qonly else 128, :],
                                                     lhsT=lhsT, rhs=wts[wname][:, kk, :],
                                                     start=(kk == 0), stop=False)
                                nc.tensor.matmul(ps[0:QS if qonly else 128, :],
                                                 lhsT=ones[0:1, 0:QS if qonly else 128],
                                                 rhs=bt[bname], start=False, stop=True)
                                yield lc, ps

                        def rope(ps, nrows, dst, cc, ss):
                            E = ps.rearrange("p (c two) -> p c two", two=2)[0:nrows, :, 0]
                            O = ps.rearrange("p (c two) -> p c two", two=2)[0:nrows, :, 1]
                            t1 = tp.tile([128, 256], FP, tag="t1")
                            t2 = tp.tile([128, 256], FP, tag="t2")
                            nc.vector.tensor_mul(t1[0:nrows], E, cc)
                            nc.vector.tensor_mul(t2[0:nrows], O, ss)
                            dv = dst.rearrange("p (h two k) -> p h two k", two=2, k=32)
                            t1r = t1[0:nrows].rearrange("p (h k) -> p h k", k=32)
                            t2r = t2[0:nrows].rearrange("p (h k) -> p h k", k=32)
                            nc.vector.tensor_sub(dv[:, :, 0, :], t1r, t2r)
                            t3 = tp.tile([128, 256], FP, tag="t1")
                            t4 = tp.tile([128, 256], FP, tag="t2")
                            nc.vector.tensor_mul(t3[0:nrows], E, ss)
                            nc.vector.tensor_mul(t4[0:nrows], O, cc)
                            nc.vector.tensor_add(dv[:, :, 1, :],
                                                 t3[0:nrows].rearrange("p (h k) -> p h k", k=32),
                                                 t4[0:nrows].rearrange("p (h k) -> p h k", k=32))

                        for b in range(B):
                            # --- Q: proj + rope + transpose -> qt_all[:, b] ---
                            qp = rp.tile([QS, D], BF, tag="qp")
                            for _, ps in proj(b, "wqT", "bq", True):
                                rope(ps, QS, qp, cosq, sinq)
                            for dc in range(NCH):
                                tps = ps_tr.tile([128, 128], BF, tag="tr")
                                nc.tensor.transpose(
                                    tps[:, 0:QS], in_=qp[:, dc * 128:(dc + 1) * 128],
                                    identity=ident[0:QS, 0:QS])
                                nc.scalar.copy(out=qt_all[:, b, dc, :], in_=tps[:, 0:QS])
                            pump(1)

                            # --- K: proj + rope + transpose -> kt ---
                            kp = rp.tile([128, NCH, D], BF, tag="kp")
                            for lc, ps in proj(b, "wkT", "bk", False):
                                rope(ps, 128, kp[:, lc], cost[:, lc, :], sint[:, lc, :])
                            kt = ktp.tile([128, NCH, L], BF, tag="kt")
                            for lc in range(NCH):
                                for dc in range(NCH):
                                    tps = ps_tr.tile([128, 128], BF, tag="tr")
                                    nc.tensor.transpose(
                                        tps, in_=kp[:, lc, dc * 128:(dc + 1) * 128],
                                        identity=ident)
                                    nc.vector.tensor_copy(
                                        out=kt[:, dc, lc * 128:(lc + 1) * 128], in_=tps)
                                pump(1 if lc % 2 == 0 else 0)

                            # --- V: proj -> va_all[:, b] (+ ones column) ---
                            nc.vector.memset(va_all[:, b, :, :, DH:DH + 1], 1.0)
                            for lc, ps in proj(b, "wvT", "bv", False):
                                nc.scalar.copy(
                                    out=va_all[:, b, lc, :, 0:DH],
                                    in_=ps.rearrange("p (h d) -> p h d", d=DH))
                            pump(1)

                            # --- scoresT + es for all heads of batch b ---
                            for h in range(H):
                                dc, po = h // 2, (h % 2) * DH
                                sps = ps_s.tile([128, NCH, QS], FP, tag="sc")
                                for m in range(NCH):
                                    nc.tensor.matmul(
                                        sps[:, m, :],
                                        lhsT=kt[po:po + DH, dc, m * 128:(m + 1) * 128],
                                        rhs=qt_all[po:po + DH, b, dc, :],
                                        start=True, stop=True,
                                        skip_group_check=True)
                                nc.scalar.activation(out=es_all[:, b, h], in_=sps,
                                                     func=AF.Exp, scale=0.125)
                            pump(1)

                        pump(NST)

                    # ---------- biasT hop: ppsum [q, k] -> ebT [k, q], exp ----------
                    with tc.tile_pool(name="hop", bufs=1) as hp, \
                         tc.tile_pool(name="ps_h", bufs=2, space="PSUM") as ph:
                        pc = hp.tile([QS, L], FP, tag="pc")
                        nc.scalar.copy(out=pc, in_=ppsum)
                        pcb = hp.tile([QS, L], BF, tag="pcb")
                        nc.vector.tensor_copy(out=pcb, in_=pc)
                        for kc in range(NCH):
                            tps = ph.tile([128, QS], BF, tag="hopt")
                            nc.tensor.transpose(
                                tps, in_=pcb[:, kc * 128:(kc + 1) * 128],
                                identity=ident[0:QS, 0:QS])
                            nc.scalar.activation(out=ebT[:, kc, :], in_=tps,
                                                 func=AF.Exp, scale=1.0 / FP8_SCALE)

                # ---------- phase B: e_t, ctx+den, normalize, out-proj ----------
                with tc.tile_pool(name="emul", bufs=4) as ep, \
                     tc.tile_pool(name="cup", bufs=2) as cup, \
                     tc.tile_pool(name="denp", bufs=2) as dp, \
                     tc.tile_pool(name="outp", bufs=2) as op_, \
                     tc.tile_pool(name="ps_ctx", bufs=4, space="PSUM") as pctx, \
                     tc.tile_pool(name="ps_bc", bufs=2, space="PSUM") as pbc, \
                     tc.tile_pool(name="ps_out", bufs=2, space="PSUM") as pout:
                    for b in range(B):
                        cu = cup.tile([DH, H, QS], BF, tag="cu")
                        den = dp.tile([1, H, QS], FP, tag="den")
                        rec = dp.tile([1, H, QS], BF, tag="rec")
                        for h in range(H):
                            e_t = ep.tile([128, NCH, QS], BF, tag="e")
                            nc.vector.tensor_mul(e_t, es_all[:, b, h], ebT)
                            cps = pctx.tile([DH + 1, QS], FP, tag="ctx")
                            for kc in range(NCH):
                                nc.tensor.matmul(
                                    cps, lhsT=va_all[:, b, kc, h, :],
                                    rhs=e_t[:, kc, :],
                                    start=(kc == 0), stop=(kc == NCH - 1),
                                    skip_group_check=True)
                            nc.scalar.copy(out=den[0:1, h, :], in_=cps[DH:DH + 1, :])
                            nc.scalar.copy(out=cu[:, h, :], in_=cps[0:DH, :])
                        nc.vector.reciprocal(den, den)
                        nc.vector.tensor_copy(out=rec, in_=den)
                        for h in range(H):
                            bps = pbc.tile([DH, QS], FP, tag="bc")
                            nc.tensor.matmul(bps, lhsT=ones[0:1, 0:DH],
                                             rhs=rec[0:1, h, :], start=True, stop=True,
                                             skip_group_check=True)
                            nc.vector.tensor_mul(cu[:, h, :], cu[:, h, :], bps)
                        ops = pout.tile([128, NCH, QS], FP, tag="ops")
                        for ch in range(NCH):
                            for h in range(H):
                                nc.tensor.matmul(
                                    ops[:, ch, :],
                                    lhsT=wo_t[:, h, ch * 128:(ch + 1) * 128],
                                    rhs=cu[:, h, :],
                                    start=(h == 0), stop=False,
                                    skip_group_check=True)
                            nc.tensor.matmul(
                                ops[:, ch, :],
                                lhsT=bt["bo"][0:1, ch * 128:(ch + 1) * 128],
                                rhs=ones[0:1, 0:QS], start=False, stop=True,
                                skip_group_check=True)
                        osb = op_.tile([128, NCH, QS], BF, tag="osb")
                        nc.scalar.copy(out=osb, in_=ops)
                        nc.sync.dma_start(
                            out=out_d[b].rearrange("(c p) q -> p c q", p=128), in_=osb)

            for _rep in range(n_repeat):
                emit(_rep == 0)
    nc.compile()
    return nc


def _pmajor(a, nch=NCH):
    """[R*128? ...] -> p-major [128, rest] image for a [(nch p), cols] tensor."""
    r, cols = a.shape
    assert r == nch * 128
    return a.reshape(nch, 128, cols).transpose(1, 0, 2).reshape(128, nch * cols)


def _in_maps(x, rel_bias, Wq, bq, Wk, bk, Wv, bv, Wo, bo):
    import ml_dtypes
    OFF = _blob_offsets()
    cos, sin = _rope_tables()
    x = np.asarray(x)
    rel_bias = np.asarray(rel_bias)

    sh = np.zeros((128, OFF["sh_total"]), dtype=np.float32)

    def put(name, img, rows=128):
        a, b_ = OFF[name]
        sh[0:rows, a:b_] = img

    xT = _f32(x.transpose(0, 2, 1))                          # [B, D, L]
    # xt section: [p, (b c l)] with d = c*128 + p
    put("xt", xT.reshape(B, NCH, 128, L).transpose(2, 0, 1, 3).reshape(128, -1))
    for nm, W in (("wqT", Wq), ("wkT", Wk), ("wvT", Wv)):
        put(nm, _pmajor(_f32(np.asarray(W).T)))
    # woT rows (h p): p-major over dh=64
    woT = _f32(np.asarray(Wo).T)
    put("woT", woT.reshape(H, DH, D).transpose(1, 0, 2).reshape(DH, H * D), DH)
    put("cost", _pmajor(_f32(cos)))
    put("sint", _pmajor(_f32(sin)))
    put("ident", np.eye(128, dtype=np.float32))
    for nm, b_ in (("bq", bq), ("bk", bk), ("bv", bv), ("bo", bo)):
        put(f"b_{nm}", _f32(np.asarray(b_)).reshape(1, D), 1)
    maps = []
    for c in range(NCORES):
        sl = rel_bias[0, c * QS:(c + 1) * QS].transpose(0, 2, 1)  # [q, d, k]
        rbp = np.ascontiguousarray(sl * FP8_SCALE).astype(ml_dtypes.float8_e4m3)
        shc = sh.copy()
        a, b_ = OFF["xq"]
        xqT = _f32(x[:, c * QS:(c + 1) * QS, :].transpose(0, 2, 1))  # [B, D, QS]
        shc[:, a:b_] = xqT.reshape(B, NCH, 128, QS).transpose(2, 0, 1, 3).reshape(128, -1)
        a, b_ = OFF["cosq"]
        shc[0:QS, a:b_] = cos[c * QS:(c + 1) * QS]
        a, b_ = OFF["sinq"]
        shc[0:QS, a:b_] = sin[c * QS:(c + 1) * QS]
        maps.append({"rb": rbp, "shblob": shc.astype(ml_dtypes.bfloat16)})
    return maps


def get_nc():
    if "nc" not in _cached:
        _cached["nc"] = _build_nc()
    return _cached["nc"]


def kernel(x, rel_bias, Wq, bq, Wk, bk, Wv, bv, Wo, bo):
    from concourse.bass_utils import run_bass_kernel_spmd
    nc = get_nc()
    maps = _in_maps(x, rel_bias, Wq, bq, Wk, bk, Wv, bv, Wo, bo)
    res = run_bass_kernel_spmd(nc, maps, core_ids=list(range(NCORES)))
    # res[c]["out"]: [B, D, QS] -> full[b, c*QS + q, :] = out[c][b, :, q].T
    out = np.concatenate(
        [np.asarray(res.results[c]["out"], dtype=np.float32).transpose(0, 2, 1)
         for c in range(NCORES)], axis=1)
    return out


# revision 5
# speedup vs baseline: 4.5016x; 1.1655x over previous
"""Trainium2 Bass kernel v2 for MultiHeadAttention with RoPE + summed relative bias.

Reference computation (B=8, L=512, D=512, H=8, dh=64):
    Q,K,V = x @ W{q,k,v}.T + b ; RoPE(Q,K) (concat variant)
    scores = Q K^T / 8 + rel_bias.sum(-1)   (bias broadcast over batch+heads)
    out = softmax(scores) V @ Wo.T + bo

Sharding (v2): core c owns QUERY rows q in [64c, 64c+64) of every batch item
and head.  The bias slice a core needs -- rel_bias.sum(-1)[qslice, :] -- is
exactly the slice it reduces locally from its 16MB fp8 [q, d, k] stream, so
there is NO collective.  K/V (and their projections + rope) are recomputed
per core for all 8 batch items; x is replicated (bf16).

All matmuls run in bf16/fp8 (fp32 PE matmuls cost 4 cycles/row).  The d-
reduction of the bias stream runs on the tensor engine directly from the
fp8 stage tiles (one-hot column selector weights land row q of PSUM), so
the stream is a plain HWDGE fp8 copy with 2KB contiguous lines.

Layouts (contraction dim on partitions):
    xT [d, l] per batch, W?T [din, dout], Q'T/K'T [d, l], scoresT/es [k, q],
    va [k, (kc, h, dh+1)] with a ones column per head (softmax denominator
    lands in PSUM row 64 of the ctx matmul), outT [dout, q] per batch
    (host transposes back).  Softmax normalization: DVE reciprocal of the
    denominator row, PE ones-broadcast, DVE multiply.
"""
import os
import numpy as np

B, L, D, H = 8, 512, 512, 8
DH = D // H          # 64
NCORES = 8
QS = L // NCORES     # 64 q rows per core
NCH = D // 128       # 4 partition chunks

_cached = {}
FP8_SCALE = 16.0


def _bf16(a):
    import ml_dtypes
    return np.ascontiguousarray(np.asarray(a, dtype=np.float32)).astype(ml_dtypes.bfloat16)


def _f32(a):
    return np.ascontiguousarray(a, dtype=np.float32)


def _rope_tables():
    freqs = (10000.0 ** (-(np.arange(0, DH, 2, dtype=np.float32) / np.float32(DH)))).astype(np.float32)
    pos = np.arange(L, dtype=np.float32)
    ang = pos[:, None] * freqs[None, :]          # [L, 32] fp32
    cos = np.cos(ang).astype(np.float32)
    sin = np.sin(ang).astype(np.float32)
    return np.tile(cos, (1, H)), np.tile(sin, (1, H))   # [L, 256]


def _blob_offsets():
    """Element offsets (bf16) of each section within the two packed blobs."""
    off, cur = {}, 0

    def add(name, n):
        nonlocal cur
        off[name] = (cur, cur + n)
        cur += n

    add("xt", B * NCH * L)            # 16384
    for nm in ("wqT", "wkT", "wvT"):
        add(nm, NCH * D)              # 2048 each
    add("woT", H * D)                 # 4096 (rows 0:64)
    add("cost", NCH * 256)            # 1024
    add("sint", NCH * 256)
    add("ident", 128)
    for nm in ("bq", "bk", "bv", "bo"):
        add(f"b_{nm}", D)             # rows 0:1
    add("xq", B * NCH * QS)           # 2048 (per-core section)
    add("cosq", 256)                  # rows 0:64 (per-core)
    add("sinq", 256)                  # rows 0:64 (per-core)
    add("rb8", QS * 4 * L // 2)       # fp8 bias stream bytes, bf16-viewed (per-core)
    off["sh_total"] = cur
    return off


def _build_nc():
    import concourse.bass as bass
    import concourse.mybir as mybir
    import concourse.tile as tile
    from concourse import bacc

    FP = mybir.dt.float32
    BF = mybir.dt.bfloat16
    F8 = mybir.dt.float8e4
    AF = mybir.ActivationFunctionType

    NQG = int(os.environ.get("MHA2_NQG", "4"))      # q rows per stage DMA
    NST = QS // NQG                                 # number of stage tiles
    stage_bufs = int(os.environ.get("MHA2_STAGE_BUFS", "5"))
    q_pre = int(os.environ.get("MHA2_QPRE", "2"))   # stages pre-pumped
    use_dr = os.environ.get("MHA2_DR", "0") == "1"  # DoubleRow reduce
    n_repeat = int(os.environ.get("MHA2_REPEAT", "1"))

    nc = bacc.Bacc(None, target_bir_lowering=False, num_devices=1)

    OFF = _blob_offsets()
    sh_d = nc.dram_tensor("shblob", [128, OFF["sh_total"]], BF, kind="ExternalInput")
    ra, rb_ = OFF["rb8"]
    # fp8 [p, (q four k)] view of the rb section: d = 4p + four
    rb_v = sh_d[:, ra:rb_].bitcast(F8).rearrange(
        "p (q four k) -> p q four k", four=4, k=L)
    out_d = nc.dram_tensor("out", [B, D, QS], BF, kind="ExternalOutput")  # outT per b

    with tile.TileContext(nc) as tc:
        with tc.tile_pool(name="persist", bufs=1) as pp, \
             tc.tile_pool(name="stage", bufs=stage_bufs) as sp:

            # ---------- persistent tiles: two mega-blobs, sliced views ----------
            ones = pp.tile([128, 128], BF)
            nc.vector.memset(ones, 1.0)
            # one-hot selector for the d-reduction: column 63 all-ones.
            # Slice [:, 63-q : 127-q] -> ones-column at local index q.
            # (memsets BEFORE the blob DMAs on the gpsimd queue)
            colones = pp.tile([128, 2 * QS - 1], F8, tag="colones")
            nc.gpsimd.memset(colones, 0.0)
            nc.gpsimd.memset(colones[:, QS - 1:QS], 1.0)
            if use_dr:
                col_dr = pp.tile([128, 2, 2 * QS], F8, tag="col_dr")
                nc.gpsimd.memset(col_dr.rearrange("p a b -> p (a b)"), 0.0)
                nc.gpsimd.memset(col_dr[:, :, QS - 1:QS], 1.0)

            sh = pp.tile([128, OFF["rb8"][0]], BF, tag="sh")
            nc.gpsimd.dma_start(out=sh, in_=sh_d[:, 0:OFF["rb8"][0]])

            def sec(name, rows=128):
                a, b_ = OFF[name]
                return sh[0:rows, a:b_]

            xt = sec("xt").rearrange("p (b c l) -> p b c l", b=B, c=NCH)
            wts = {nm: sec(nm).rearrange("p (c j) -> p c j", c=NCH)
                   for nm in ("wqT", "wkT", "wvT")}
            wo_t = sec("woT", DH).rearrange("p (h j) -> p h j", h=H)
            cost = sec("cost").rearrange("p (c k) -> p c k", c=NCH)
            sint = sec("sint").rearrange("p (c k) -> p c k", c=NCH)
            ident = sec("ident")
            bt = {nm: sec(f"b_{nm}", 1) for nm in ("bq", "bk", "bv", "bo")}
            xq = sec("xq").rearrange("p (b c l) -> p b c l", b=B, c=NCH)
            cosq = sec("cosq", QS)
            sinq = sec("sinq", QS)

            es_all = pp.tile([128, B, H, NCH, QS], BF, tag="es_all")  # exp(scoresT/8)
            ebT = pp.tile([128, NCH, QS], BF, tag="ebT")              # exp(biasT)
            qt_all = pp.tile([128, B, NCH, QS], BF, tag="qt")         # Q'T [d, q]
            va_all = pp.tile([128, B, NCH, H, DH + 1], BF, tag="va")  # V [k, ...]+ones

            def emit(first):
                # ---------- bias stream: fp8 [q, d, k] -> PE reduce ----------
                with tc.tile_pool(name="ppsum", bufs=1, space="PSUM") as ppp:
                    ppsum = ppp.tile([QS, L], FP, tag="ppsum")

                    def stream_stage(si):
                        st = sp.tile([128, NQG, 4, L], F8, tag="stage")
                        dma_eng = nc.sync if si % 2 == 0 else nc.scalar
                        dma_eng.dma_start(
                            out=st, in_=rb_v[:, si * NQG:(si + 1) * NQG])
                        for qq in range(NQG):
                            q = si * NQG + qq
                            if use_dr:
                                for jj in range(2):
                                    nc.tensor.matmul(
                                        ppsum,
                                        lhsT=col_dr[:, :, QS - 1 - q:2 * QS - 1 - q],
                                        rhs=st[:, qq, 2 * jj:2 * jj + 2, :],
                                        start=(q == 0 and jj == 0),
                                        stop=(q == QS - 1 and jj == 1),
                                        perf_mode=mybir.MatmulPerfMode.DoubleRow,
                                        skip_group_check=True)
                            else:
                                for jj in range(4):
                                    nc.tensor.matmul(
                                        ppsum,
                                        lhsT=colones[:, QS - 1 - q:2 * QS - 1 - q],
                                        rhs=st[:, qq, jj, :],
                                        start=(q == 0 and jj == 0),
                                        stop=(q == QS - 1 and jj == 3),
                                        skip_group_check=True)

                    st_it = iter(range(NST))

                    def pump(n):
                        for _ in range(n):
                            si = next(st_it, None)
                            if si is None:
                                return
                            stream_stage(si)

                    pump(q_pre)

                    # ---------- phase A: projections + rope + scoresT + es ----------
                    with tc.tile_pool(name="rope", bufs=2) as rp, \
                         tc.tile_pool(name="ktp", bufs=2) as ktp, \
                         tc.tile_pool(name="ps_a", bufs=3, space="PSUM") as ps_a, \
                         tc.tile_pool(name="ps_tr", bufs=2, space="PSUM") as ps_tr, \
                         tc.tile_pool(name="ps_s", bufs=2, space="PSUM") as ps_s, \
                         tc.tile_pool(name="tmp", bufs=4) as tp:

                        def proj(b, wname, bname, qonly):
                            """PSUM [l-rows, 512 dout] for one l-chunk (gen)."""
                            nlc = 1 if qonly else NCH
                            for lc in range(nlc):
                                ps = ps_a.tile([128, D], FP, tag="proj")
                                for kk in range(NCH):
                                    lhsT = (xq[:, b, kk, :] if qonly
                                            else xt[:, b, kk, lc * 128:(lc + 1) * 128])
                                    nc.tensor.matmul(ps[0:QS if qonly else 128, :],
                                                     lhsT=lhsT, rhs=wts[wname][:, kk, :],
                                                     start=(kk == 0), stop=False)
                                nc.tensor.matmul(ps[0:QS if qonly else 128, :],
                                                 lhsT=ones[0:1, 0:QS if qonly else 128],
                                                 rhs=bt[bname], start=False, stop=True)
                                yield lc, ps

                        def rope(ps, nrows, dst, cc, ss):
                            E = ps.rearrange("p (c two) -> p c two", two=2)[0:nrows, :, 0]
                            O = ps.rearrange("p (c two) -> p c two", two=2)[0:nrows, :, 1]
                            t1 = tp.tile([128, 256], FP, tag="t1")
                            t2 = tp.tile([128, 256], FP, tag="t2")
                            nc.vector.tensor_mul(t1[0:nrows], E, cc)
                            nc.vector.tensor_mul(t2[0:nrows], O, ss)
                            dv = dst.rearrange("p (h two k) -> p h two k", two=2, k=32)
                            t1r = t1[0:nrows].rearrange("p (h k) -> p h k", k=32)
                            t2r = t2[0:nrows].rearrange("p (h k) -> p h k", k=32)
                            nc.vector.tensor_sub(dv[:, :, 0, :], t1r, t2r)
                            t3 = tp.tile([128, 256], FP, tag="t1")
                            t4 = tp.tile([128, 256], FP, tag="t2")
                            nc.vector.tensor_mul(t3[0:nrows], E, ss)
                            nc.vector.tensor_mul(t4[0:nrows], O, cc)
                            nc.vector.tensor_add(dv[:, :, 1, :],
                                                 t3[0:nrows].rearrange("p (h k) -> p h k", k=32),
                                                 t4[0:nrows].rearrange("p (h k) -> p h k", k=32))

                        for b in range(B):
                            # --- Q: proj + rope + transpose -> qt_all[:, b] ---
                            qp = rp.tile([QS, D], BF, tag="qp")
                            for _, ps in proj(b, "wqT", "bq", True):
                                rope(ps, QS, qp, cosq, sinq)
                            for dc in range(NCH):
                                tps = ps_tr.tile([128, 128], BF, tag="tr")
                                nc.tensor.transpose(
                                    tps[:, 0:QS], in_=qp[:, dc * 128:(dc + 1) * 128],
                                    identity=ident[0:QS, 0:QS])
                                nc.scalar.copy(out=qt_all[:, b, dc, :], in_=tps[:, 0:QS])
                            pump(1)

                            # --- K: proj + rope + transpose -> kt ---
                            kp = rp.tile([128, NCH, D], BF, tag="kp")
                            for lc, ps in proj(b, "wkT", "bk", False):
                                rope(ps, 128, kp[:, lc], cost[:, lc, :], sint[:, lc, :])
                            kt = ktp.tile([128, NCH, L], BF, tag="kt")
                            for lc in range(NCH):
                                for dc in range(NCH):
                                    tps = ps_tr.tile([128, 128], BF, tag="tr")
                                    nc.tensor.transpose(
                                        tps, in_=kp[:, lc, dc * 128:(dc + 1) * 128],
                                        identity=ident)
                                    nc.vector.tensor_copy(
                                        out=kt[:, dc, lc * 128:(lc + 1) * 128], in_=tps)
                                pump(1 if lc % 2 == 0 else 0)

                            # --- V: proj -> va_all[:, b] (+ ones column) ---
                            nc.vector.memset(va_all[:, b, :, :, DH:DH + 1], 1.0)
                            for lc, ps in proj(b, "wvT", "bv", False):
                                nc.scalar.copy(
                                    out=va_all[:, b, lc, :, 0:DH],
                                    in_=ps.rearrange("p (h d) -> p h d", d=DH))
                            pump(1)

                            # --- scoresT + es for all heads of batch b ---
                            for h in range(H):
                                dc, po = h // 2, (h % 2) * DH
                                sps = ps_s.tile([128, NCH, QS], FP, tag="sc")
                                for m in range(NCH):
                                    nc.tensor.matmul(
                                        sps[:, m, :],
                                        lhsT=kt[po:po + DH, dc, m * 128:(m + 1) * 128],
                                        rhs=qt_all[po:po + DH, b, dc, :],
                                        start=True, stop=True,
                                        skip_group_check=True)
                                nc.scalar.activation(out=es_all[:, b, h], in_=sps,
                                                     func=AF.Exp, scale=0.125)
                            pump(1)

                        pump(NST)

                    # ---------- biasT hop: ppsum [q, k] -> ebT [k, q], exp ----------
                    with tc.tile_pool(name="hop", bufs=1) as hp, \
                         tc.tile_pool(name="ps_h", bufs=2, space="PSUM") as ph:
                        pc = hp.tile([QS, L], FP, tag="pc")
                        nc.scalar.copy(out=pc, in_=ppsum)
                        pcb = hp.tile([QS, L], BF, tag="pcb")
                        nc.vector.tensor_copy(out=pcb, in_=pc)
                        for kc in range(NCH):
                            tps = ph.tile([128, QS], BF, tag="hopt")
                            nc.tensor.transpose(
                                tps, in_=pcb[:, kc * 128:(kc + 1) * 128],
                                identity=ident[0:QS, 0:QS])
                            nc.scalar.activation(out=ebT[:, kc, :], in_=tps,
                                                 func=AF.Exp, scale=1.0 / FP8_SCALE)

                # ---------- phase B: e_t, ctx+den, normalize, out-proj ----------
                with tc.tile_pool(name="emul", bufs=4) as ep, \
                     tc.tile_pool(name="cup", bufs=2) as cup, \
                     tc.tile_pool(name="denp", bufs=2) as dp, \
                     tc.tile_pool(name="outp", bufs=2) as op_, \
                     tc.tile_pool(name="ps_ctx", bufs=4, space="PSUM") as pctx, \
                     tc.tile_pool(name="ps_bc", bufs=2, space="PSUM") as pbc, \
                     tc.tile_pool(name="ps_out", bufs=2, space="PSUM") as pout:
                    for b in range(B):
                        cu = cup.tile([DH, H, QS], BF, tag="cu")
                        den = dp.tile([1, H, QS], FP, tag="den")
                        rec = dp.tile([1, H, QS], BF, tag="rec")
                        for h in range(H):
                            e_t = ep.tile([128, NCH, QS], BF, tag="e")
                            nc.vector.tensor_mul(e_t, es_all[:, b, h], ebT)
                            cps = pctx.tile([DH + 1, QS], FP, tag="ctx")
                            for kc in range(NCH):
                                nc.tensor.matmul(
                                    cps, lhsT=va_all[:, b, kc, h, :],
                                    rhs=e_t[:, kc, :],
                                    start=(kc == 0), stop=(kc == NCH - 1),
                                    skip_group_check=True)
                            nc.scalar.copy(out=den[0:1, h, :], in_=cps[DH:DH + 1, :])
                            nc.scalar.copy(out=cu[:, h, :], in_=cps[0:DH, :])
                        nc.vector.reciprocal(den, den)
                        nc.vector.tensor_copy(out=rec, in_=den)
                        for h in range(H):
                            bps = pbc.tile([DH, QS], FP, tag="bc")
                            nc.tensor.matmul(bps, lhsT=ones[0:1, 0:DH],
                                             rhs=rec[0:1, h, :], start=True, stop=True,
                                             skip_group_check=True)
                            nc.vector.tensor_mul(cu[:, h, :], cu[:, h, :], bps)
                        ops = pout.tile([128, NCH, QS], FP, tag="ops")
                        for ch in range(NCH):
                            for h in range(H):
                                nc.tensor.matmul(
                                    ops[:, ch, :],
                                    lhsT=wo_t[:, h, ch * 128:(ch + 1) * 128],
                                    rhs=cu[:, h, :],
                                    start=(h == 0), stop=False,
                                    skip_group_check=True)
                            nc.tensor.matmul(
                                ops[:, ch, :],
                                lhsT=bt["bo"][0:1, ch * 128:(ch + 1) * 128],
                                rhs=ones[0:1, 0:QS], start=False, stop=True,
                                skip_group_check=True)
                        osb = op_.tile([128, NCH, QS], BF, tag="osb")
                        nc.scalar.copy(out=osb, in_=ops)
                        nc.sync.dma_start(
                            out=out_d[b].rearrange("(c p) q -> p c q", p=128), in_=osb)

            for _rep in range(n_repeat):
                emit(_rep == 0)
    nc.compile()
    return nc


def _pmajor(a, nch=NCH):
    """[R*128? ...] -> p-major [128, rest] image for a [(nch p), cols] tensor."""
    r, cols = a.shape
    assert r == nch * 128
    return a.reshape(nch, 128, cols).transpose(1, 0, 2).reshape(128, nch * cols)


def _in_maps(x, rel_bias, Wq, bq, Wk, bk, Wv, bv, Wo, bo):
    import ml_dtypes
    OFF = _blob_offsets()
    cos, sin = _rope_tables()
    x = np.asarray(x)
    rel_bias = np.asarray(rel_bias)

    sh = np.zeros((128, OFF["sh_total"]), dtype=np.float32)

    def put(name, img, rows=128):
        a, b_ = OFF[name]
        sh[0:rows, a:b_] = img

    xT = _f32(x.transpose(0, 2, 1))                          # [B, D, L]
    # xt section: [p, (b c l)] with d = c*128 + p
    put("xt", xT.reshape(B, NCH, 128, L).transpose(2, 0, 1, 3).reshape(128, -1))
    for nm, W in (("wqT", Wq), ("wkT", Wk), ("wvT", Wv)):
        put(nm, _pmajor(_f32(np.asarray(W).T)))
    # woT rows (h p): p-major over dh=64
    woT = _f32(np.asarray(Wo).T)
    put("woT", woT.reshape(H, DH, D).transpose(1, 0, 2).reshape(DH, H * D), DH)
    put("cost", _pmajor(_f32(cos)))
    put("sint", _pmajor(_f32(sin)))
    put("ident", np.eye(128, dtype=np.float32))
    for nm, b_ in (("bq", bq), ("bk", bk), ("bv", bv), ("bo", bo)):
        put(f"b_{nm}", _f32(np.asarray(b_)).reshape(1, D), 1)
    maps = []
    for c in range(NCORES):
        sl = rel_bias[0, c * QS:(c + 1) * QS].transpose(0, 2, 1)  # [q, d, k]
        rbp = np.ascontiguousarray(sl * FP8_SCALE).astype(ml_dtypes.float8_e4m3)
        # p-major stream image: [p, (q, four, k)] with d = 4p + four
        rb_img = rbp.reshape(QS, 128, 4, L).transpose(1, 0, 2, 3).reshape(128, -1)
        shc = sh.copy()
        a, b_ = OFF["xq"]
        xqT = _f32(x[:, c * QS:(c + 1) * QS, :].transpose(0, 2, 1))  # [B, D, QS]
        shc[:, a:b_] = xqT.reshape(B, NCH, 128, QS).transpose(2, 0, 1, 3).reshape(128, -1)
        a, b_ = OFF["cosq"]
        shc[0:QS, a:b_] = cos[c * QS:(c + 1) * QS]
        a, b_ = OFF["sinq"]
        shc[0:QS, a:b_] = sin[c * QS:(c + 1) * QS]
        shc_bf = shc.astype(ml_dtypes.bfloat16)
        a, b_ = OFF["rb8"]
        shc_bf[:, a:b_] = np.ascontiguousarray(rb_img).view(ml_dtypes.bfloat16)
        maps.append({"shblob": shc_bf})
    return maps


def get_nc():
    if "nc" not in _cached:
        _cached["nc"] = _build_nc()
    return _cached["nc"]


def kernel(x, rel_bias, Wq, bq, Wk, bk, Wv, bv, Wo, bo):
    from concourse.bass_utils import run_bass_kernel_spmd
    nc = get_nc()
    maps = _in_maps(x, rel_bias, Wq, bq, Wk, bk, Wv, bv, Wo, bo)
    res = run_bass_kernel_spmd(nc, maps, core_ids=list(range(NCORES)))
    # res[c]["out"]: [B, D, QS] -> full[b, c*QS + q, :] = out[c][b, :, q].T
    out = np.concatenate(
        [np.asarray(res.results[c]["out"], dtype=np.float32).transpose(0, 2, 1)
         for c in range(NCORES)], axis=1)
    return out


# revision 6
# speedup vs baseline: 5.8549x; 1.3006x over previous
"""Trainium2 Bass kernel v2 for MultiHeadAttention with RoPE + summed relative bias.

Reference computation (B=8, L=512, D=512, H=8, dh=64):
    Q,K,V = x @ W{q,k,v}.T + b ; RoPE(Q,K) (concat variant)
    scores = Q K^T / 8 + rel_bias.sum(-1)   (bias broadcast over batch+heads)
    out = softmax(scores) V @ Wo.T + bo

Sharding (v2): core c owns QUERY rows q in [64c, 64c+64) of every batch item
and head.  The bias slice a core needs -- rel_bias.sum(-1)[qslice, :] -- is
exactly the slice it reduces locally from its 16MB fp8 [q, d, k] stream, so
there is NO collective.  K/V (and their projections + rope) are recomputed
per core for all 8 batch items; x is replicated (bf16).

All matmuls run in bf16/fp8 (fp32 PE matmuls cost 4 cycles/row).  The d-
reduction of the bias stream runs on the tensor engine directly from the
fp8 stage tiles (one-hot column selector weights land row q of PSUM), so
the stream is a plain HWDGE fp8 copy with 2KB contiguous lines.

Layouts (contraction dim on partitions):
    xT [d, l] per batch, W?T [din, dout], Q'T/K'T [d, l], scoresT/es [k, q],
    va [k, (kc, h, dh+1)] with a ones column per head (softmax denominator
    lands in PSUM row 64 of the ctx matmul), outT [dout, q] per batch
    (host transposes back).  Softmax normalization: DVE reciprocal of the
    denominator row, PE ones-broadcast, DVE multiply.
"""
import os
import numpy as np

B, L, D, H = 8, 512, 512, 8
DH = D // H          # 64
NCORES = 8
QS = L // NCORES     # 64 q rows per core
NCH = D // 128       # 4 partition chunks

_cached = {}
FP8_SCALE = 16.0


def _bf16(a):
    import ml_dtypes
    return np.ascontiguousarray(np.asarray(a, dtype=np.float32)).astype(ml_dtypes.bfloat16)


def _f32(a):
    return np.ascontiguousarray(a, dtype=np.float32)


def _rope_tables():
    freqs = (10000.0 ** (-(np.arange(0, DH, 2, dtype=np.float32) / np.float32(DH)))).astype(np.float32)
    pos = np.arange(L, dtype=np.float32)
    ang = pos[:, None] * freqs[None, :]          # [L, 32] fp32
    cos = np.cos(ang).astype(np.float32)
    sin = np.sin(ang).astype(np.float32)
    return np.tile(cos, (1, H)), np.tile(sin, (1, H))   # [L, 256]


def _blob_offsets():
    """Element offsets (bf16) of each section within the two packed blobs."""
    off, cur = {}, 0

    def add(name, n):
        nonlocal cur
        off[name] = (cur, cur + n)
        cur += n

    add("xt", B * NCH * L)            # 16384
    for nm in ("wqT", "wkT", "wvT"):
        add(nm, NCH * D)              # 2048 each
    add("woT", H * D)                 # 4096 (rows 0:64)
    add("cost", NCH * 256)            # 1024
    add("sint", NCH * 256)
    add("ident", 128)
    for nm in ("bq", "bk", "bv", "bo"):
        add(f"b_{nm}", D)             # rows 0:1
    add("xq", B * NCH * QS)           # 2048 (per-core section)
    add("cosq", 256)                  # rows 0:64 (per-core)
    add("sinq", 256)                  # rows 0:64 (per-core)
    add("rb8", QS * 4 * L // 2)       # fp8 bias stream bytes, bf16-viewed (per-core)
    off["sh_total"] = cur
    return off


def _build_nc():
    import concourse.bass as bass
    import concourse.mybir as mybir
    import concourse.tile as tile
    from concourse import bacc

    FP = mybir.dt.float32
    BF = mybir.dt.bfloat16
    F8 = mybir.dt.float8e4
    AF = mybir.ActivationFunctionType

    NQG = int(os.environ.get("MHA2_NQG", "4"))      # q rows per stage DMA
    NST = QS // NQG                                 # number of stage tiles
    stage_bufs = int(os.environ.get("MHA2_STAGE_BUFS", "5"))
    q_pre = int(os.environ.get("MHA2_QPRE", "2"))   # stages pre-pumped
    use_dr = os.environ.get("MHA2_DR", "0") == "1"  # DoubleRow reduce
    n_repeat = int(os.environ.get("MHA2_REPEAT", "1"))

    nc = bacc.Bacc(None, target_bir_lowering=False, num_devices=1,
                   enable_partition_id=False)

    OFF = _blob_offsets()
    sh_d = nc.dram_tensor("shblob", [128, OFF["sh_total"]], BF, kind="ExternalInput")
    ra, rb_ = OFF["rb8"]
    # fp8 [p, (q four k)] view of the rb section: d = 4p + four
    rb_v = sh_d[:, ra:rb_].bitcast(F8).rearrange(
        "p (q four k) -> p q four k", four=4, k=L)
    out_d = nc.dram_tensor("out", [B, D, QS], BF, kind="ExternalOutput")  # outT per b

    with tile.TileContext(nc) as tc:
        with tc.tile_pool(name="persist", bufs=1) as pp, \
             tc.tile_pool(name="stage", bufs=stage_bufs) as sp:

            # ---------- persistent tiles: two mega-blobs, sliced views ----------
            ones = pp.tile([128, 128], BF)
            nc.vector.memset(ones, 1.0)
            # one-hot selector for the d-reduction: column 63 all-ones.
            # Slice [:, 63-q : 127-q] -> ones-column at local index q.
            # (memsets BEFORE the blob DMAs on the gpsimd queue)
            colones = pp.tile([128, 2 * QS - 1], F8, tag="colones")
            nc.gpsimd.memset(colones, 0.0)
            nc.gpsimd.memset(colones[:, QS - 1:QS], 1.0)
            if use_dr:
                col_dr = pp.tile([128, 2, 2 * QS], F8, tag="col_dr")
                nc.gpsimd.memset(col_dr.rearrange("p a b -> p (a b)"), 0.0)
                nc.gpsimd.memset(col_dr[:, :, QS - 1:QS], 1.0)

            sh = pp.tile([128, OFF["rb8"][0]], BF, tag="sh")
            nc.gpsimd.dma_start(out=sh, in_=sh_d[:, 0:OFF["rb8"][0]])

            def sec(name, rows=128):
                a, b_ = OFF[name]
                return sh[0:rows, a:b_]

            xt = sec("xt").rearrange("p (b c l) -> p b c l", b=B, c=NCH)
            wts = {nm: sec(nm).rearrange("p (c j) -> p c j", c=NCH)
                   for nm in ("wqT", "wkT", "wvT")}
            wo_t = sec("woT", DH).rearrange("p (h j) -> p h j", h=H)
            cost = sec("cost").rearrange("p (c k) -> p c k", c=NCH)
            sint = sec("sint").rearrange("p (c k) -> p c k", c=NCH)
            ident = sec("ident")
            bt = {nm: sec(f"b_{nm}", 1) for nm in ("bq", "bk", "bv", "bo")}
            xq = sec("xq").rearrange("p (b c l) -> p b c l", b=B, c=NCH)
            cosq = sec("cosq", QS)
            sinq = sec("sinq", QS)

            es_all = pp.tile([128, B, H, NCH, QS], BF, tag="es_all")  # exp(scoresT/8)
            ebT = pp.tile([128, NCH, QS], BF, tag="ebT")              # exp(biasT)
            qt_all = pp.tile([128, B, NCH, QS], BF, tag="qt")         # Q'T [d, q]
            va_all = pp.tile([128, B, NCH, H, DH + 1], BF, tag="va")  # V [k, ...]+ones

            def emit(first):
                # ---------- bias stream: fp8 [q, d, k] -> PE reduce ----------
                with tc.tile_pool(name="ppsum", bufs=1, space="PSUM") as ppp:
                    ppsum = ppp.tile([QS, L], FP, tag="ppsum")

                    def stream_stage(si):
                        st = sp.tile([128, NQG, 4, L], F8, tag="stage")
                        dma_eng = nc.sync if si % 2 == 0 else nc.scalar
                        dma_eng.dma_start(
                            out=st, in_=rb_v[:, si * NQG:(si + 1) * NQG])
                        for qq in range(NQG):
                            q = si * NQG + qq
                            if use_dr:
                                for jj in range(2):
                                    nc.tensor.matmul(
                                        ppsum,
                                        lhsT=col_dr[:, :, QS - 1 - q:2 * QS - 1 - q],
                                        rhs=st[:, qq, 2 * jj:2 * jj + 2, :],
                                        start=(q == 0 and jj == 0),
                                        stop=(q == QS - 1 and jj == 1),
                                        perf_mode=mybir.MatmulPerfMode.DoubleRow,
                                        skip_group_check=True)
                            else:
                                for jj in range(4):
                                    nc.tensor.matmul(
                                        ppsum,
                                        lhsT=colones[:, QS - 1 - q:2 * QS - 1 - q],
                                        rhs=st[:, qq, jj, :],
                                        start=(q == 0 and jj == 0),
                                        stop=(q == QS - 1 and jj == 3),
                                        skip_group_check=True)

                    st_it = iter(range(NST))

                    def pump(n):
                        for _ in range(n):
                            si = next(st_it, None)
                            if si is None:
                                return
                            stream_stage(si)

                    pump(q_pre)

                    # ---------- phase A: projections + rope + scoresT + es ----------
                    with tc.tile_pool(name="rope", bufs=2) as rp, \
                         tc.tile_pool(name="ktp", bufs=2) as ktp, \
                         tc.tile_pool(name="ps_a", bufs=3, space="PSUM") as ps_a, \
                         tc.tile_pool(name="ps_tr", bufs=2, space="PSUM") as ps_tr, \
                         tc.tile_pool(name="ps_s", bufs=2, space="PSUM") as ps_s, \
                         tc.tile_pool(name="tmp", bufs=4) as tp:

                        def proj(b, wname, bname, qonly):
                            """PSUM [l-rows, 512 dout] for one l-chunk (gen)."""
                            nlc = 1 if qonly else NCH
                            for lc in range(nlc):
                                ps = ps_a.tile([128, D], FP, tag="proj")
                                for kk in range(NCH):
                                    lhsT = (xq[:, b, kk, :] if qonly
                                            else xt[:, b, kk, lc * 128:(lc + 1) * 128])
                                    nc.tensor.matmul(ps[0:QS if qonly else 128, :],
                                                     lhsT=lhsT, rhs=wts[wname][:, kk, :],
                                                     start=(kk == 0), stop=False)
                                nc.tensor.matmul(ps[0:QS if qonly else 128, :],
                                                 lhsT=ones[0:1, 0:QS if qonly else 128],
                                                 rhs=bt[bname], start=False, stop=True)
                                yield lc, ps

                        def rope(ps, nrows, dst, cc, ss):
                            E = ps.rearrange("p (c two) -> p c two", two=2)[0:nrows, :, 0]
                            O = ps.rearrange("p (c two) -> p c two", two=2)[0:nrows, :, 1]
                            t1 = tp.tile([128, 256], FP, tag="t1")
                            t2 = tp.tile([128, 256], FP, tag="t2")
                            nc.vector.tensor_mul(t1[0:nrows], E, cc)
                            nc.vector.tensor_mul(t2[0:nrows], O, ss)
                            dv = dst.rearrange("p (h two k) -> p h two k", two=2, k=32)
                            t1r = t1[0:nrows].rearrange("p (h k) -> p h k", k=32)
                            t2r = t2[0:nrows].rearrange("p (h k) -> p h k", k=32)
                            nc.vector.tensor_sub(dv[:, :, 0, :], t1r, t2r)
                            t3 = tp.tile([128, 256], FP, tag="t1")
                            t4 = tp.tile([128, 256], FP, tag="t2")
                            nc.vector.tensor_mul(t3[0:nrows], E, ss)
                            nc.vector.tensor_mul(t4[0:nrows], O, cc)
                            nc.vector.tensor_add(dv[:, :, 1, :],
                                                 t3[0:nrows].rearrange("p (h k) -> p h k", k=32),
                                                 t4[0:nrows].rearrange("p (h k) -> p h k", k=32))

                        for b in range(B):
                            # --- Q: proj + rope + transpose -> qt_all[:, b] ---
                            qp = rp.tile([QS, D], BF, tag="qp")
                            for _, ps in proj(b, "wqT", "bq", True):
                                rope(ps, QS, qp, cosq, sinq)
                            for dc in range(NCH):
                                tps = ps_tr.tile([128, 128], BF, tag="tr")
                                nc.tensor.transpose(
                                    tps[:, 0:QS], in_=qp[:, dc * 128:(dc + 1) * 128],
                                    identity=ident[0:QS, 0:QS])
                                nc.scalar.copy(out=qt_all[:, b, dc, :], in_=tps[:, 0:QS])
                            pump(1)

                            # --- K: proj + rope + transpose -> kt ---
                            kp = rp.tile([128, NCH, D], BF, tag="kp")
                            for lc, ps in proj(b, "wkT", "bk", False):
                                rope(ps, 128, kp[:, lc], cost[:, lc, :], sint[:, lc, :])
                            kt = ktp.tile([128, NCH, L], BF, tag="kt")
                            for lc in range(NCH):
                                for dc in range(NCH):
                                    tps = ps_tr.tile([128, 128], BF, tag="tr")
                                    nc.tensor.transpose(
                                        tps, in_=kp[:, lc, dc * 128:(dc + 1) * 128],
                                        identity=ident)
                                    nc.vector.tensor_copy(
                                        out=kt[:, dc, lc * 128:(lc + 1) * 128], in_=tps)
                                pump(1 if lc % 2 == 0 else 0)

                            # --- V: proj -> va_all[:, b] (+ ones column) ---
                            nc.vector.memset(va_all[:, b, :, :, DH:DH + 1], 1.0)
                            for lc, ps in proj(b, "wvT", "bv", False):
                                nc.scalar.copy(
                                    out=va_all[:, b, lc, :, 0:DH],
                                    in_=ps.rearrange("p (h d) -> p h d", d=DH))
                            pump(1)

                            # --- scoresT + es for all heads of batch b ---
                            for h in range(H):
                                dc, po = h // 2, (h % 2) * DH
                                sps = ps_s.tile([128, NCH, QS], FP, tag="sc")
                                for m in range(NCH):
                                    nc.tensor.matmul(
                                        sps[:, m, :],
                                        lhsT=kt[po:po + DH, dc, m * 128:(m + 1) * 128],
                                        rhs=qt_all[po:po + DH, b, dc, :],
                                        start=True, stop=True,
                                        skip_group_check=True)
                                nc.scalar.activation(out=es_all[:, b, h], in_=sps,
                                                     func=AF.Exp, scale=0.125)
                            pump(1)

                        pump(NST)

                    # ---------- biasT hop: ppsum [q, k] -> ebT [k, q], exp ----------
                    with tc.tile_pool(name="hop", bufs=1) as hp, \
                         tc.tile_pool(name="ps_h", bufs=2, space="PSUM") as ph:
                        pc = hp.tile([QS, L], FP, tag="pc")
                        nc.scalar.copy(out=pc, in_=ppsum)
                        pcb = hp.tile([QS, L], BF, tag="pcb")
                        nc.vector.tensor_copy(out=pcb, in_=pc)
                        for kc in range(NCH):
                            tps = ph.tile([128, QS], BF, tag="hopt")
                            nc.tensor.transpose(
                                tps, in_=pcb[:, kc * 128:(kc + 1) * 128],
                                identity=ident[0:QS, 0:QS])
                            nc.scalar.activation(out=ebT[:, kc, :], in_=tps,
                                                 func=AF.Exp, scale=1.0 / FP8_SCALE)

                # ---------- phase B: e_t, ctx+den, normalize, out-proj ----------
                with tc.tile_pool(name="emul", bufs=4) as ep, \
                     tc.tile_pool(name="cup", bufs=2) as cup, \
                     tc.tile_pool(name="denp", bufs=2) as dp, \
                     tc.tile_pool(name="outp", bufs=2) as op_, \
                     tc.tile_pool(name="ps_ctx", bufs=4, space="PSUM") as pctx, \
                     tc.tile_pool(name="ps_bc", bufs=2, space="PSUM") as pbc, \
                     tc.tile_pool(name="ps_out", bufs=2, space="PSUM") as pout:
                    for b in range(B):
                        cu = cup.tile([DH, H, QS], BF, tag="cu")
                        den = dp.tile([1, H, QS], FP, tag="den")
                        rec = dp.tile([1, H, QS], BF, tag="rec")
                        for h in range(H):
                            e_t = ep.tile([128, NCH, QS], BF, tag="e")
                            nc.vector.tensor_mul(e_t, es_all[:, b, h], ebT)
                            cps = pctx.tile([DH + 1, QS], FP, tag="ctx")
                            for kc in range(NCH):
                                nc.tensor.matmul(
                                    cps, lhsT=va_all[:, b, kc, h, :],
                                    rhs=e_t[:, kc, :],
                                    start=(kc == 0), stop=(kc == NCH - 1),
                                    skip_group_check=True)
                            nc.scalar.copy(out=den[0:1, h, :], in_=cps[DH:DH + 1, :])
                            nc.scalar.copy(out=cu[:, h, :], in_=cps[0:DH, :])
                        nc.vector.reciprocal(den, den)
                        nc.vector.tensor_copy(out=rec, in_=den)
                        for h in range(H):
                            bps = pbc.tile([DH, QS], FP, tag="bc")
                            nc.tensor.matmul(bps, lhsT=ones[0:1, 0:DH],
                                             rhs=rec[0:1, h, :], start=True, stop=True,
                                             skip_group_check=True)
                            nc.vector.tensor_mul(cu[:, h, :], cu[:, h, :], bps)
                        ops = pout.tile([128, NCH, QS], FP, tag="ops")
                        for ch in range(NCH):
                            for h in range(H):
                                nc.tensor.matmul(
                                    ops[:, ch, :],
                                    lhsT=wo_t[:, h, ch * 128:(ch + 1) * 128],
                                    rhs=cu[:, h, :],
                                    start=(h == 0), stop=False,
                                    skip_group_check=True)
                            nc.tensor.matmul(
                                ops[:, ch, :],
                                lhsT=bt["bo"][0:1, ch * 128:(ch + 1) * 128],
                                rhs=ones[0:1, 0:QS], start=False, stop=True,
                                skip_group_check=True)
                        osb = op_.tile([128, NCH, QS], BF, tag="osb")
                        nc.scalar.copy(out=osb, in_=ops)
                        nc.sync.dma_start(
                            out=out_d[b].rearrange("(c p) q -> p c q", p=128), in_=osb)

            for _rep in range(n_repeat):
                emit(_rep == 0)
    nc.compile()
    return nc


def _pmajor(a, nch=NCH):
    """[R*128? ...] -> p-major [128, rest] image for a [(nch p), cols] tensor."""
    r, cols = a.shape
    assert r == nch * 128
    return a.reshape(nch, 128, cols).transpose(1, 0, 2).reshape(128, nch * cols)


def _in_maps(x, rel_bias, Wq, bq, Wk, bk, Wv, bv, Wo, bo):
    import ml_dtypes
    OFF = _blob_offsets()
    cos, sin = _rope_tables()
    x = np.asarray(x)
    rel_bias = np.asarray(rel_bias)

    sh = np.zeros((128, OFF["sh_total"]), dtype=np.float32)

    def put(name, img, rows=128):
        a, b_ = OFF[name]
        sh[0:rows, a:b_] = img

    xT = _f32(x.transpose(0, 2, 1))                          # [B, D, L]
    # xt section: [p, (b c l)] with d = c*128 + p
    put("xt", xT.reshape(B, NCH, 128, L).transpose(2, 0, 1, 3).reshape(128, -1))
    for nm, W in (("wqT", Wq), ("wkT", Wk), ("wvT", Wv)):
        put(nm, _pmajor(_f32(np.asarray(W).T)))
    # woT rows (h p): p-major over dh=64
    woT = _f32(np.asarray(Wo).T)
    put("woT", woT.reshape(H, DH, D).transpose(1, 0, 2).reshape(DH, H * D), DH)
    put("cost", _pmajor(_f32(cos)))
    put("sint", _pmajor(_f32(sin)))
    put("ident", np.eye(128, dtype=np.float32))
    for nm, b_ in (("bq", bq), ("bk", bk), ("bv", bv), ("bo", bo)):
        put(f"b_{nm}", _f32(np.asarray(b_)).reshape(1, D), 1)
    maps = []
    for c in range(NCORES):
        sl = rel_bias[0, c * QS:(c + 1) * QS].transpose(0, 2, 1)  # [q, d, k]
        rbp = np.ascontiguousarray(sl * FP8_SCALE).astype(ml_dtypes.float8_e4m3)
        # p-major stream image: [p, (q, four, k)] with d = 4p + four
        rb_img = rbp.reshape(QS, 128, 4, L).transpose(1, 0, 2, 3).reshape(128, -1)
        shc = sh.copy()
        a, b_ = OFF["xq"]
        xqT = _f32(x[:, c * QS:(c + 1) * QS, :].transpose(0, 2, 1))  # [B, D, QS]
        shc[:, a:b_] = xqT.reshape(B, NCH, 128, QS).transpose(2, 0, 1, 3).reshape(128, -1)
        a, b_ = OFF["cosq"]
        shc[0:QS, a:b_] = cos[c * QS:(c + 1) * QS]
        a, b_ = OFF["sinq"]
        shc[0:QS, a:b_] = sin[c * QS:(c + 1) * QS]
        shc_bf = shc.astype(ml_dtypes.bfloat16)
        a, b_ = OFF["rb8"]
        shc_bf[:, a:b_] = np.ascontiguousarray(rb_img).view(ml_dtypes.bfloat16)
        maps.append({"shblob": shc_bf})
    return maps


def get_nc():
    if "nc" not in _cached:
        _cached["nc"] = _build_nc()
    return _cached["nc"]


def kernel(x, rel_bias, Wq, bq, Wk, bk, Wv, bv, Wo, bo):
    from concourse.bass_utils import run_bass_kernel_spmd
    nc = get_nc()
    maps = _in_maps(x, rel_bias, Wq, bq, Wk, bk, Wv, bv, Wo, bo)
    res = run_bass_kernel_spmd(nc, maps, core_ids=list(range(NCORES)))
    # res[c]["out"]: [B, D, QS] -> full[b, c*QS + q, :] = out[c][b, :, q].T
    out = np.concatenate(
        [np.asarray(res.results[c]["out"], dtype=np.float32).transpose(0, 2, 1)
         for c in range(NCORES)], axis=1)
    return out
